# revision 12
# baseline (speedup 1.0000x reference)
"""Trainium2 Bass kernel for the EntityBert CRF loss (llh + viterbi decode).

kernel(**inputs) takes FULL inputs (B=256,L=512,H=768,C=9), shards batch
across 8 NeuronCores, runs one SPMD Bass/Tile program per core, returns
(llh_scalar_f32, decoded_int32[B,L]) matching reference semantics.

Algorithm per core (NB=32 examples):
  em = feats @ W + b  (PE, fp32, t-windowed so scans can pipeline)
  forward logsumexp scan in exp space, G-normalized, chunked over L in 4
    chunks on (b,chunk)=128 partitions with provably-contracting warmup;
    per-chunk mass chained; logZ gathered at len-1 via host one-hots.
  viterbi forward scan (scores only) same chunking; backpointers extracted
    in a batch pass; backtrace via exact map-composition over chunks;
    decode selected by entry tags.
  gold-path numerator via host-precomputed int-derived one-hot weights.
Host does sharding + int-derived constant prep only; all float math on device.
"""
import os
import sys
from contextlib import ExitStack

import numpy as np

sys.path.insert(0, "/opt/trn_rl_repo")

import concourse.bass as bass
import concourse.bacc as bacc
import concourse.tile as tile
from concourse import mybir
from concourse.bass_utils import run_bass_kernel_spmd

f32 = np.float32

B, L, H, C = 256, 512, 768, 9
M = 8                 # cores
NB = B // M           # 32 examples per core
NCH = 4               # L-chunks for the scans
CH = L // NCH         # 128
WARM = 16             # warmup steps for chunk entries
C2 = C * C            # 81
KCH = H // 128        # 6 K-chunks
TW = 16               # t-win per matmul chunk
NW = L // TW          # 32 matmul chunks
ENC = f32(16.0)       # tag encoding: enc = 16 - c

DT = mybir.dt.float32
DI = mybir.dt.int32
DU = mybir.dt.uint32

DEBUG = bool(int(os.environ.get("KERNEL_DEBUG", "0")))
TRACE = bool(int(os.environ.get("KERNEL_TRACE", "0")))

A = mybir.AluOpType
AF = mybir.ActivationFunctionType
AX = mybir.AxisListType


def _bc(ap, shape):
    return ap.to_broadcast(shape)


def build_program():
    nc = bacc.Bacc("TRN2", target_bir_lowering=False, debug=False)

    def din(name, shape, dt=DT):
        return nc.dram_tensor(name, list(shape), dt, kind="ExternalInput").ap()

    featsT = din("featsT", [H, L * NB])          # [h, w*512 + b*16 + t4]
    w_re = din("w_re", [128, KCH * C])
    bff = din("bff", [C, 1])
    tt81 = din("tt81", [128, C2])                # T[c,j] at j*9+c
    ett81 = din("ett81", [128, C2])              # exp(T)[c,j] at j*9+c
    tflat = din("tflat", [NB, C2])               # T[c,c'] at c*9+c'
    iota81 = din("iota81", [128, C2])            # c at (j*9+c)
    iotaenc = din("iotaenc", [128, C])           # 16 - c at col c
    expend = din("expend", [128, C])
    endr = din("endr", [128, C])
    startr = din("startr", [128, C])
    expstart = din("expstart", [128, C])
    ones32 = din("ones32", [NB, 1])
    onesN = din("onesN", [1, 512])
    ohL = din("ohL", [128, CH])
    ohLany = din("ohLany", [128, 1])
    mInv = din("mInv", [128, CH], DU)             # 1 where t=ch*128+i' is MASKED-OFF
    mAf = din("mAf", [128, CH])                   # mask (t<len) f32 at t=ch*128+i
    ch0m = din("ch0m", [128, 1], DU)
    rowm = [din(f"rowm{k}", [128, 1], DU) for k in range(NCH)]
    w1 = din("w1", [NB, C * L])
    counts = din("counts", [NB, C2])
    ohst = din("ohst", [NB, C])
    ohen = din("ohen", [NB, C])

    o_llh = nc.dram_tensor("llh_part", [1, 1], DT, kind="ExternalOutput").ap()
    o_dec = nc.dram_tensor("decoded", [NB, L], DI, kind="ExternalOutput").ap()
    dbg = {}
    if DEBUG:
        for n, s in dict(em_scan=[128, C * CH], G=[128, CH], LGc=[128, CH],
                         phist=[128, CH * C], D=[128, CH], logZ=[NB, NCH],
                         shist=[128, CH * C], bph=[128, CH * C],
                         traj=[128, CH * C], lastenc=[NB, 1], num=[NB, 1],
                         decf=[128, CH]).items():
            dbg[n] = nc.dram_tensor("dbg_" + n, s, DT, kind="ExternalOutput").ap()

    with tile.TileContext(nc) as tc, ExitStack() as ctx:
        pool = ctx.enter_context(tc.tile_pool(name="main", bufs=1))
        fpool = ctx.enter_context(tc.tile_pool(name="feats", bufs=3))
        ppool = ctx.enter_context(tc.tile_pool(name="ps", bufs=4, space="PSUM"))
        dpool = ctx.enter_context(tc.tile_pool(name="dsc", bufs=1, space="DRAM"))

        def const_tile(ap, shape, dt=DT, tag=None):
            t = pool.tile(shape, dt, tag=tag or ap.tensor.name)
            nc.sync.dma_start(t[:], ap)
            return t

        w_t = const_tile(w_re, [128, KCH * C])
        bff_t = const_tile(bff, [C, 1])
        tt81_t = const_tile(tt81, [128, C2])
        ett81_t = const_tile(ett81, [128, C2])
        tflat_t = const_tile(tflat, [NB, C2])
        io81_t = const_tile(iota81, [128, C2])
        ioenc_t = const_tile(iotaenc, [128, C])
        expend_t = const_tile(expend, [128, C])
        endr_t = const_tile(endr, [128, C])
        startr_t = const_tile(startr, [128, C])
        expstart_t = const_tile(expstart, [128, C])
        ones32_t = const_tile(ones32, [NB, 1])
        onesN_t = const_tile(onesN, [1, 512])
        ohL_t = const_tile(ohL, [128, CH])
        ohLany_t = const_tile(ohLany, [128, 1])
        mInv_t = const_tile(mInv, [128, CH], DU)
        mAf_t = const_tile(mAf, [128, CH])
        ch0m_t = const_tile(ch0m, [128, 1], DU)
        rowm_t = [const_tile(rowm[k], [128, 1], DU) for k in range(NCH)]
        w1_t = const_tile(w1, [NB, C * L])
        counts_t = const_tile(counts, [NB, C2])
        ohst_t = const_tile(ohst, [NB, C])
        ohen_t = const_tile(ohen, [NB, C])

        # ---- P1: em matmul ----
        # PSUM chunk [c(9), (b,t4)]  ->  DRAM scratch in both target layouts
        d_scan = dpool.tile([128, C * CH], DT, tag="d_scan")   # p=(ch,b), c*128+i
        d_gold = dpool.tile([NB, C * L], DT, tag="d_gold")     # b, c*512+t
        for w in range(NW):
            ps = ppool.tile([C, 512], DT, tag="emps")
            for k in range(KCH):
                fch = fpool.tile([128, 512], DT, tag="fch")
                nc.sync.dma_start(fch[:], featsT[128 * k:128 * (k + 1),
                                                 512 * w:512 * (w + 1)])
                nc.tensor.matmul(ps[:], w_t[:, C * k:C * (k + 1)], fch[:],
                                 start=(k == 0), stop=(k == KCH - 1))
            em_cb = fpool.tile([C, 512], DT, tag="em_cb")
            nc.scalar.activation(em_cb[:], ps[:], AF.Identity, bias=bff_t[:])
            ch, iw = w // 8, w % 8
            # src iter (c, b, t4); dst d_scan[(ch*32+b), c*128 + iw*16 + t4]
            nc.sync.dma_start(
                d_scan[:].rearrange("(ch b) (c n i) -> ch b c n i",
                                    ch=NCH, c=C, n=8)[ch, :, :, iw, :]
                .transpose((1, 0, 2)),
                em_cb[:].rearrange("c (b t) -> c b t", b=NB))
            # dst d_gold[b, c*512 + w*16 + t4]
            nc.sync.dma_start(
                d_gold[:].rearrange("b (c w t) -> b c w t", c=C, w=NW)[:, :, w, :]
                .transpose((1, 0, 2)),
                em_cb[:].rearrange("c (b t) -> c b t", b=NB))

        em_scan = pool.tile([128, C * CH], DT)
        nc.sync.dma_start(em_scan[:], d_scan[:])
        em_gold = pool.tile([NB, C * L], DT)
        nc.sync.dma_start(em_gold[:], d_gold[:])
        if DEBUG:
            nc.sync.dma_start(dbg["em_scan"], em_scan[:])

        def ic(t, n=CH):   # [p, i, c] view of a [p, c*n] (c-outer) tile
            return t[:].rearrange("p (c i) -> p i c", c=C)

        # ---- P2: E, G, Etil, logG, LGc ----
        E_t = pool.tile([128, C * CH], DT)
        nc.scalar.activation(E_t[:], em_scan[:], AF.Exp)
        G_t = pool.tile([128, CH], DT)
        nc.vector.tensor_reduce(G_t[:], ic(E_t), AX.X, A.add)
        rG_t = pool.tile([128, CH], DT)
        nc.vector.reciprocal(rG_t[:], G_t[:])
        Etil = pool.tile([128, C * CH], DT)
        nc.vector.tensor_tensor(ic(Etil), ic(E_t),
                                rG_t[:].unsqueeze(2).to_broadcast((128, CH, C)),
                                A.mult)
        logG = pool.tile([128, CH], DT)
        nc.scalar.activation(logG[:], G_t[:], AF.Ln)
        zCH = pool.tile([128, CH], DT)
        nc.vector.memset(zCH[:], 0.0)
        LGc = pool.tile([128, CH], DT)
        nc.vector.tensor_tensor_scan(LGc[:], logG[:], zCH[:], 0.0, A.add, A.add)
        if DEBUG:
            nc.sync.dma_start(dbg["G"], G_t[:])
            nc.sync.dma_start(dbg["LGc"], LGc[:])

        # warmup slices from previous chunk (partition shift by -32)
        EtW = pool.tile([128, C * WARM], DT)
        nc.sync.dma_start(
            EtW[32:128, :].rearrange("p (c q) -> p c q", c=C),
            Etil[0:96, :].rearrange("p (c i) -> p c i", c=C)[:, :, CH - WARM:])
        nc.vector.memset(EtW[0:32, :], 1.0)
        emW = pool.tile([128, C * WARM], DT)
        nc.sync.dma_start(
            emW[32:128, :].rearrange("p (c q) -> p c q", c=C),
            em_scan[0:96, :].rearrange("p (c i) -> p c i", c=C)[:, :, CH - WARM:])
        nc.vector.memset(emW[0:32, :], 0.0)

        # ---------- P3: forward exp-space scan ----------
        ph = pool.tile([128, CH * C], DT)          # free i*9+c
        st = pool.tile([128, C], DT)
        tmp81 = pool.tile([128, C2], DT, tag="tmp81")
        nc.vector.memset(st[:], 1.0 / C)

        def fwd_step(dst_ap, etil_slice):
            nc.vector.tensor_tensor(
                tmp81[:].rearrange("p (j c) -> p j c", j=C),
                st[:].unsqueeze(1).to_broadcast((128, C, C)),
                ett81_t[:].rearrange("p (j c) -> p j c", j=C), A.mult)
            v9 = pool.tile([128, C], DT, tag="v9f")
            nc.vector.tensor_reduce(v9[:], tmp81[:].rearrange("p (j c) -> p j c", j=C),
                                    AX.X, A.add)
            nc.vector.tensor_tensor(dst_ap, v9[:], etil_slice, A.mult)

        EtW_v = EtW[:].rearrange("p (c q) -> p q c", c=C)
        for q in range(WARM):
            fwd_step(st[:], EtW_v[:, q, :])
        ms = pool.tile([128, 1], DT)
        nc.vector.tensor_reduce(ms[:], st[:], AX.X, A.add)
        rms = pool.tile([128, 1], DT)
        nc.vector.reciprocal(rms[:], ms[:])
        nc.vector.tensor_scalar(st[:], st[:], rms[:], None, A.mult)

        Et_ic = ic(Etil)
        ph_ic = ph[:].rearrange("p (i c) -> p i c", i=CH)
        fwd_step(ph_ic[:, 0, :], Et_ic[:, 0, :])
        init0 = pool.tile([128, C], DT)
        nc.vector.tensor_tensor(init0[:], expstart_t[:], Et_ic[:, 0, :], A.mult)
        nc.vector.copy_predicated(ph_ic[:, 0, :], _bc(ch0m_t[:], (128, C)), init0[:])
        nc.vector.tensor_copy(st[:], ph_ic[:, 0, :])
        for i in range(1, CH):
            fwd_step(ph_ic[:, i, :], Et_ic[:, i, :])
            nc.vector.tensor_copy(st[:], ph_ic[:, i, :])
        if DEBUG:
            nc.sync.dma_start(dbg["phist"], ph[:])

        # ---------- P4: logZ ----------
        Dw = pool.tile([128, CH * C], DT, tag="Dw")
        nc.vector.tensor_tensor(Dw[:].rearrange("p (i c) -> p i c", i=CH), ph_ic,
                                expend_t[:].unsqueeze(1).to_broadcast((128, CH, C)),
                                A.mult)
        D_t = pool.tile([128, CH], DT)
        nc.vector.tensor_reduce(D_t[:], Dw[:].rearrange("p (i c) -> p i c", i=CH),
                                AX.X, A.add)
        logD = pool.tile([128, CH], DT)
        nc.scalar.activation(logD[:], D_t[:], AF.Ln)
        Gval = pool.tile([128, CH], DT)
        nc.vector.tensor_tensor(Gval[:], logD[:], LGc[:], A.add)
        zsel = pool.tile([128, CH], DT, tag="zsel")
        nc.vector.tensor_tensor(zsel[:], Gval[:], ohL_t[:], A.mult)
        zred = pool.tile([128, 1], DT)
        nc.vector.tensor_reduce(zred[:], zsel[:], AX.X, A.add)
        pm = pool.tile([128, 1], DT)
        nc.vector.tensor_reduce(pm[:], ph_ic[:, CH - 1, :], AX.X, A.add)
        lpm = pool.tile([128, 1], DT)
        nc.scalar.activation(lpm[:], pm[:], AF.Ln)
        LM = pool.tile([128, 1], DT)
        nc.vector.tensor_tensor(LM[:], lpm[:], LGc[:, CH - 1:CH], A.add)
        # regroup [128,1] -> [NB, NCH] via DRAM
        d_lm = dpool.tile([128, 1], DT, tag="d_lm")
        nc.sync.dma_start(d_lm[:], LM[:])
        LMb = pool.tile([NB, NCH], DT)
        nc.sync.dma_start(LMb[:], d_lm[:].rearrange("(k b) o -> b k o", k=NCH))
        LMi = pool.tile([NB, NCH], DT)
        zN = pool.tile([NB, NCH], DT, tag="zN")
        nc.vector.memset(zN[:], 0.0)
        nc.vector.tensor_tensor_scan(LMi[:], LMb[:], zN[:], 0.0, A.add, A.add)
        kap = pool.tile([NB, NCH], DT)
        nc.vector.tensor_tensor(kap[:], LMi[:], LMb[:], A.subtract)
        d_kap = dpool.tile([128, 1], DT, tag="d_kap")
        nc.sync.dma_start(d_kap[:].rearrange("(k b) o -> b k o", k=NCH), kap[:])
        kap128 = pool.tile([128, 1], DT)
        nc.sync.dma_start(kap128[:], d_kap[:])
        kapsel = pool.tile([128, 1], DT)
        nc.vector.tensor_tensor(kapsel[:], kap128[:], ohLany_t[:, 0:1], A.mult)
        zk = pool.tile([128, 1], DT)
        nc.vector.tensor_tensor(zk[:], zred[:], kapsel[:], A.add)
        d_zk = dpool.tile([128, 1], DT, tag="d_zk")
        nc.sync.dma_start(d_zk[:], zk[:])
        zkb = pool.tile([NB, NCH], DT)
        nc.sync.dma_start(zkb[:], d_zk[:].rearrange("(k b) o -> b k o", k=NCH))
        logZ = pool.tile([NB, 1], DT)
        nc.vector.tensor_reduce(logZ[:], zkb[:], AX.X, A.add)
        if DEBUG:
            nc.sync.dma_start(dbg["D"], D_t[:])
            nc.sync.dma_start(dbg["logZ"], zkb[:])

        # ---------- P5: viterbi forward scan (scores only, exact sequential) ----------
        # [NB partitions]; em slices come from em_gold[b, (c, t)] strided reads.
        shs = pool.tile([NB, L * C], DT)           # s-hist, free t*9+c
        shs_tc = shs[:].rearrange("b (t c) -> b t c", t=L)
        emg_ct = em_gold[:].rearrange("b (c t) -> b t c", c=C)
        tmp81b = pool.tile([NB, C2], DT, tag="tmp81b")
        sv = pool.tile([NB, C], DT)
        nc.vector.tensor_tensor(shs_tc[:, 0, :], startr_t[0:NB, :], emg_ct[:, 0, :],
                                A.add)
        nc.vector.tensor_copy(sv[:], shs_tc[:, 0, :])
        for t in range(1, L):
            nc.vector.tensor_tensor(
                tmp81b[:].rearrange("p (j c) -> p j c", j=C),
                sv[:].unsqueeze(1).to_broadcast((NB, C, C)),
                tt81_t[0:NB, :].rearrange("p (j c) -> p j c", j=C), A.add)
            mx = pool.tile([NB, C], DT, tag="mxv")
            nc.vector.tensor_reduce(mx[:], tmp81b[:].rearrange("p (j c) -> p j c", j=C),
                                    AX.X, A.max)
            dst = shs_tc[:, t, :]
            nc.vector.tensor_tensor(dst, mx[:], emg_ct[:, t, :], A.add)
            nc.vector.tensor_copy(sv[:], dst)
        # rearrange s-hist to the (b,ch)-partition layout via DRAM
        d_sh = dpool.tile([128, CH * C], DT, tag="d_sh")
        nc.sync.dma_start(
            d_sh[:].rearrange("(k b) f -> b k f", k=NCH), shs[:])
        sh = pool.tile([128, CH * C], DT)
        nc.sync.dma_start(sh[:], d_sh[:])
        sh_ic = sh[:].rearrange("p (i c) -> p i c", i=CH)
        if DEBUG:
            nc.sync.dma_start(dbg["shist"], sh[:])

        # ---------- P6: batch bp extraction (enc space) ----------
        sprev0 = pool.tile([128, C], DT)
        nc.sync.dma_start(sprev0[32:128, :], sh[0:96, (CH - 1) * C:])
        nc.vector.memset(sprev0[0:32, :], 0.0)
        bph = pool.tile([128, CH * C], DT)         # bp_enc at free i*9+j
        BLK = 32

        def bp_block(sprev_ap, dst_ap, n):
            cb = pool.tile([128, BLK * C2], DT, tag="cb")
            cb4 = cb[:, 0:n * C2].rearrange("p (i j c) -> p i j c", j=C, c=C)
            nc.vector.tensor_tensor(
                cb4, sprev_ap.unsqueeze(2).to_broadcast((128, n, C, C)),
                tt81_t[:].rearrange("p (j c) -> p j c", j=C)
                .unsqueeze(1).to_broadcast((128, n, C, C)), A.add)
            mxb = pool.tile([128, BLK * C], DT, tag="mxb")
            mxb3 = mxb[:, 0:n * C].rearrange("p (i j) -> p i j", j=C)
            nc.vector.tensor_reduce(mxb3, cb4, AX.X, A.max)
            nc.vector.tensor_tensor(cb4, cb4,
                                    mxb3.unsqueeze(3).to_broadcast((128, n, C, C)),
                                    A.is_ge)
            eqb = pool.tile([128, BLK * C2], DT, tag="eqb")
            eqb4 = eqb[:, 0:n * C2].rearrange("p (i j c) -> p i j c", j=C, c=C)
            nc.vector.scalar_tensor_tensor(
                eqb4, cb4, float(ENC),
                io81_t[:].rearrange("p (j c) -> p j c", j=C)
                .unsqueeze(1).to_broadcast((128, n, C, C)),
                A.mult, A.subtract)
            nc.vector.tensor_reduce(dst_ap, eqb4, AX.X, A.max)

        bp_ij = bph[:].rearrange("p (i j) -> p i j", i=CH)
        bp_block(sprev0[:].unsqueeze(1), bp_ij[:, 0:1, :], 1)
        i = 1
        while i < CH:
            n = min(BLK, CH - i)
            bp_block(sh_ic[:, i - 1:i - 1 + n, :], bp_ij[:, i:i + n, :], n)
            i += n
        if DEBUG:
            nc.sync.dma_start(dbg["bph"], bph[:])

        bpx = pool.tile([128, C], DT)
        nc.sync.dma_start(bpx[0:96, :], bph[32:128, 0:C])
        nc.vector.memset(bpx[96:128, :], 0.0)

        # ---------- P7: backtrace phase A ----------
        traj = pool.tile([128, CH * C], DT)
        mapst = pool.tile([128, C], DT)
        nc.vector.tensor_copy(mapst[:], ioenc_t[:])
        eq81 = pool.tile([128, C2], DT, tag="eq81")
        tr_ij = traj[:].rearrange("p (i j) -> p i j", i=CH)
        for ip in range(CH, 0, -1):
            bsl = bpx[:] if ip == CH else bp_ij[:, ip, :]
            nc.vector.tensor_tensor(
                eq81[:].rearrange("p (j m) -> p j m", j=C),
                mapst[:].unsqueeze(2).to_broadcast((128, C, C)),
                ioenc_t[:].unsqueeze(1).to_broadcast((128, C, C)), A.is_equal)
            g81 = pool.tile([128, C2], DT, tag="g81")
            nc.vector.tensor_tensor(
                g81[:].rearrange("p (j m) -> p j m", j=C),
                eq81[:].rearrange("p (j m) -> p j m", j=C),
                bsl.unsqueeze(1).to_broadcast((128, C, C)), A.mult)
            dst = tr_ij[:, ip - 1, :]
            nc.vector.tensor_reduce(dst, g81[:].rearrange("p (j m) -> p j m", j=C),
                                    AX.X, A.add)
            nc.vector.copy_predicated(dst, _bc(mInv_t[:, ip - 1:ip], (128, C)),
                                      mapst[:])
            nc.vector.tensor_copy(mapst[:], dst)
        if DEBUG:
            nc.sync.dma_start(dbg["traj"], traj[:])

        # ---------- P8: last tag + entry chaining + decode ----------
        sselw = pool.tile([128, CH * C], DT, tag="sselw")
        nc.vector.tensor_tensor(sselw[:].rearrange("p (i c) -> p i c", i=CH), sh_ic,
                                ohL_t[:].unsqueeze(2).to_broadcast((128, CH, C)),
                                A.mult)
        ssel = pool.tile([128, C], DT)
        nc.vector.tensor_reduce(ssel[:],
                                sselw[:].rearrange("p (i c) -> p c i", i=CH),
                                AX.X, A.add)
        d_ss = dpool.tile([128, C], DT, tag="d_ss")
        nc.sync.dma_start(d_ss[:], ssel[:])
        sselb = pool.tile([NB, NCH * C], DT)
        nc.sync.dma_start(sselb[:], d_ss[:].rearrange("(k b) c -> b k c", k=NCH))
        sfin = pool.tile([NB, C], DT)
        nc.vector.tensor_reduce(sfin[:],
                                sselb[:].rearrange("b (k c) -> b c k", k=NCH),
                                AX.X, A.add)
        nc.vector.tensor_tensor(sfin[:], sfin[:], endr_t[0:NB, :], A.add)
        mxf = pool.tile([NB, 1], DT)
        nc.vector.tensor_reduce(mxf[:], sfin[:], AX.X, A.max)
        eqf = pool.tile([NB, C], DT)
        nc.vector.tensor_scalar(eqf[:], sfin[:], mxf[:], None, A.is_ge)
        vf = pool.tile([NB, C], DT)
        nc.vector.scalar_tensor_tensor(vf[:], eqf[:], float(ENC), io81_t[0:NB, 0:C],
                                       A.mult, A.subtract)
        lastenc = pool.tile([NB, 1], DT)
        nc.vector.tensor_reduce(lastenc[:], vf[:], AX.X, A.max)
        if DEBUG:
            nc.sync.dma_start(dbg["lastenc"], lastenc[:])

        ent128 = pool.tile([128, 1], DT)
        nc.vector.memset(ent128[:], 0.0)
        decf = pool.tile([128, CH], DT)
        entb = pool.tile([NB, 1], DT, tag="entb")
        nc.vector.tensor_copy(entb[:], lastenc[:])
        for k in range(NCH - 1, -1, -1):
            nc.sync.dma_start(ent128[32 * k:32 * (k + 1), :], entb[:])
            ohe = pool.tile([128, C], DT, tag="ohe")
            nc.vector.tensor_scalar(ohe[:], ioenc_t[:], ent128[:], None, A.is_equal)
            selw = pool.tile([128, CH * C], DT, tag="selw")
            nc.vector.tensor_tensor(
                selw[:].rearrange("p (i j) -> p i j", i=CH), tr_ij,
                ohe[:].unsqueeze(1).to_broadcast((128, CH, C)), A.mult)
            dsel = pool.tile([128, CH], DT, tag="dsel")
            nc.vector.tensor_reduce(dsel[:],
                                    selw[:].rearrange("p (i j) -> p i j", i=CH),
                                    AX.X, A.add)
            nc.vector.copy_predicated(decf[:], _bc(rowm_t[k][:], (128, CH)), dsel[:])
            nc.sync.dma_start(entb[:], dsel[32 * k:32 * (k + 1), 0:1])
        nc.vector.tensor_scalar(decf[:], decf[:], float(-ENC), -1.0, A.add, op1=A.mult)
        nc.vector.tensor_tensor(decf[:], decf[:], mAf_t[:], A.mult)
        if DEBUG:
            nc.sync.dma_start(dbg["decf"], decf[:])
        deci = pool.tile([128, CH], DI)
        nc.vector.tensor_copy(deci[:], decf[:])
        for k in range(NCH):
            nc.sync.dma_start(o_dec[:, CH * k:CH * (k + 1)],
                              deci[32 * k:32 * (k + 1), :])

        # ---------- P9: gold numerator + llh partial ----------
        junk = pool.tile([NB, C * L], DT, tag="junk")
        emp = pool.tile([NB, 1], DT)
        nc.vector.scalar_tensor_tensor(junk[:], em_gold[:], 1.0, w1_t[:],
                                       A.mult, A.mult, accum_out=emp[:])
        tw_ = pool.tile([NB, C2], DT, tag="tw_")
        nc.vector.tensor_tensor(tw_[:], counts_t[:], tflat_t[:], A.mult)
        tp = pool.tile([NB, 1], DT)
        nc.vector.tensor_reduce(tp[:], tw_[:], AX.X, A.add)
        sw_ = pool.tile([NB, C], DT, tag="sw_")
        nc.vector.tensor_tensor(sw_[:], ohst_t[:], startr_t[0:NB, :], A.mult)
        sp = pool.tile([NB, 1], DT)
        nc.vector.tensor_reduce(sp[:], sw_[:], AX.X, A.add)
        ew_ = pool.tile([NB, C], DT, tag="ew_")
        nc.vector.tensor_tensor(ew_[:], ohen_t[:], endr_t[0:NB, :], A.mult)
        ep = pool.tile([NB, 1], DT)
        nc.vector.tensor_reduce(ep[:], ew_[:], AX.X, A.add)
        num = pool.tile([NB, 1], DT)
        nc.vector.tensor_tensor(num[:], emp[:], tp[:], A.add)
        nc.vector.tensor_tensor(num[:], num[:], sp[:], A.add)
        nc.vector.tensor_tensor(num[:], num[:], ep[:], A.add)
        if DEBUG:
            nc.sync.dma_start(dbg["num"], num[:])
        diff = pool.tile([NB, 1], DT)
        nc.vector.tensor_tensor(diff[:], num[:], logZ[:], A.subtract)
        acc_ps = ppool.tile([1, 1], DT, tag="accps")
        nc.tensor.matmul(acc_ps[:], diff[:], ones32_t[:], start=True, stop=True)
        acc = pool.tile([1, 1], DT)
        nc.scalar.copy(acc[:], acc_ps[:])
        nc.sync.dma_start(o_llh, acc[:])

    nc.compile()
    return nc


def host_prep(inputs):
    feats = np.asarray(inputs["feats"], f32)
    W = np.asarray(inputs["W_ff"], f32)
    b_ff = np.asarray(inputs["b_ff"], f32)
    start = np.asarray(inputs["start_transitions"], f32)
    end = np.asarray(inputs["end_transitions"], f32)
    T = np.asarray(inputs["transitions"], f32)
    tags = np.asarray(inputs["tags"])
    lengths = np.asarray(inputs["lengths"])

    expT = np.exp(T).astype(f32)
    rep = lambda a: np.ascontiguousarray(
        np.tile(np.asarray(a, f32).reshape(1, -1), (128, 1)))
    tt81 = rep(T.T.reshape(-1))
    ett81 = rep(expT.T.reshape(-1))
    iota81 = rep(np.tile(np.arange(C, dtype=f32), C))
    iotaenc = rep(ENC - np.arange(C, dtype=f32))
    expend = rep(np.exp(end))
    endr = rep(end)
    startr = rep(start)
    expstart = rep(np.exp(start))
    w_re = np.ascontiguousarray(
        W.reshape(KCH, 128, C).transpose(1, 0, 2).reshape(128, KCH * C)).astype(f32)

    shared = dict(
        w_re=w_re, bff=np.ascontiguousarray(b_ff.reshape(C, 1)).astype(f32),
        tt81=tt81, ett81=ett81, iota81=iota81, iotaenc=iotaenc,
        expend=expend, endr=endr, startr=startr, expstart=expstart,
        tflat=np.ascontiguousarray(np.tile(T.reshape(1, -1), (NB, 1))).astype(f32),
        ones32=np.ones((NB, 1), f32), onesN=np.ones((1, 512), f32),
    )

    in_maps = []
    bidx = np.arange(NB)
    for c in range(M):
        sl = slice(c * NB, (c + 1) * NB)
        fe = feats[sl]
        tg = tags[sl].astype(np.int64)
        ln = lengths[sl].astype(np.int64)
        fw = fe.reshape(NB, NW, TW, H).transpose(3, 1, 0, 2).reshape(H, L * NB)
        featsT = np.ascontiguousarray(fw).astype(f32)

        m = (np.arange(L)[None, :] < ln[:, None])
        ohL = np.zeros((128, CH), f32)
        ohLany = np.zeros((128, 1), f32)
        mInv = np.zeros((128, CH), np.uint32)
        mAf = np.zeros((128, CH), f32)
        ch0m = np.zeros((128, 1), np.uint32)
        ch0m[0:32] = 1
        rowms = []
        for ch in range(NCH):
            t0 = ch * CH
            for b in range(NB):
                p = 32 * ch + b
                lm1 = int(ln[b]) - 1
                if t0 <= lm1 < t0 + CH:
                    ohL[p, lm1 - t0] = 1.0
                    ohLany[p, 0] = 1.0
                tv = np.arange(t0 + 1, t0 + CH + 1)
                mInv[p, :] = (tv >= ln[b]).astype(np.uint32)
                mAf[p, :] = m[b, t0:t0 + CH].astype(f32)
            rm = np.zeros((128, 1), np.uint32)
            rm[32 * ch:32 * (ch + 1)] = 1
            rowms.append(rm)
        w1 = np.zeros((NB, C, L), f32)
        for t in range(L):
            w1[bidx, tg[:, t], t] = m[:, t].astype(f32)
        counts = np.zeros((NB, C2), f32)
        for b in range(NB):
            for t in range(1, int(ln[b])):
                counts[b, tg[b, t - 1] * C + tg[b, t]] += 1
        ohst = np.zeros((NB, C), f32)
        ohst[bidx, tg[:, 0]] = 1
        ohen = np.zeros((NB, C), f32)
        ohen[bidx, tg[bidx, ln - 1]] = 1

        im = dict(shared)
        im.update(featsT=featsT, ohL=ohL, ohLany=ohLany, mInv=mInv, mAf=mAf,
                  ch0m=ch0m, w1=np.ascontiguousarray(w1.reshape(NB, C * L)),
                  counts=counts, ohst=ohst, ohen=ohen)
        for k in range(NCH):
            im[f"rowm{k}"] = rowms[k]
        in_maps.append(im)
    return in_maps


_prog_cache = {}


def get_program():
    if "nc" not in _prog_cache:
        _prog_cache["nc"] = build_program()
    return _prog_cache["nc"]


def _install_ntff_hook():
    """Provide antenv.axon_hooks via ctypes on images that lack it."""
    import types
    import ctypes
    import contextlib
    try:
        from antenv.axon_hooks import get_axon_ntff_profile_hook  # noqa: F401
        return
    except ImportError:
        pass
    try:
        lib = ctypes.CDLL("/opt/axon/libaxon_pjrt.so")
        if not hasattr(lib, "axon_start_nrt_profile"):
            return
    except OSError:
        return
    lib.axon_start_nrt_profile.argtypes = [ctypes.POINTER(ctypes.c_int64),
                                           ctypes.c_size_t]
    lib.axon_start_nrt_profile.restype = ctypes.c_int64
    lib.axon_stop_nrt_profile.argtypes = [ctypes.c_char_p]
    lib.axon_stop_nrt_profile.restype = ctypes.c_int64

    @contextlib.contextmanager
    def _hook(output_dir, device_ids):
        import jax
        jax.devices()
        if device_ids:
            ids = (ctypes.c_int64 * len(device_ids))(*device_ids)
            rc = lib.axon_start_nrt_profile(ids, len(device_ids))
        else:
            rc = lib.axon_start_nrt_profile(None, 0)
        if rc != 0:
            raise RuntimeError(f"axon_start_nrt_profile rc={rc}")
        try:
            yield
        finally:
            n = lib.axon_stop_nrt_profile(str(output_dir).encode())
            print(f"ntff profile: {n} file(s) -> {output_dir}")

    mod = types.ModuleType("antenv.axon_hooks")
    mod.get_axon_ntff_profile_hook = lambda: _hook
    mod.set_axon_ntff_profile_hook = lambda h: None
    sys.modules["antenv.axon_hooks"] = mod


def kernel(**inputs):
    nc = get_program()
    in_maps = host_prep(inputs)
    if TRACE:
        _install_ntff_hook()
    res = run_bass_kernel_spmd(nc, in_maps, list(range(M)), trace=TRACE)
    llh = np.sum([r["llh_part"][0, 0] for r in res.results], dtype=f32) / f32(B)
    decoded = np.concatenate([r["decoded"] for r in res.results], 0).astype(np.int32)
    kernel.last_results = res
    kernel.last_exec_time_ns = getattr(res, "exec_time_ns", None)
    return np.float32(llh), decoded


# revision 14
# speedup vs baseline: 1.3386x; 1.3386x over previous
"""Trainium2 Bass kernel for the EntityBert CRF loss (llh + viterbi decode).

kernel(**inputs) takes FULL inputs (B=256,L=512,H=768,C=9), shards batch
across 8 NeuronCores, runs one SPMD Bass/Tile program per core, returns
(llh_scalar_f32, decoded_int32[B,L]) matching reference semantics.

Algorithm per core (NB=32 examples):
  em = feats @ W + b  (PE, fp32, t-windowed so scans can pipeline)
  forward logsumexp scan in exp space, G-normalized, chunked over L in 4
    chunks on (b,chunk)=128 partitions with provably-contracting warmup;
    per-chunk mass chained; logZ gathered at len-1 via host one-hots.
  viterbi forward scan (scores only) same chunking; backpointers extracted
    in a batch pass; backtrace via exact map-composition over chunks;
    decode selected by entry tags.
  gold-path numerator via host-precomputed int-derived one-hot weights.
Host does sharding + int-derived constant prep only; all float math on device.
"""
import os
import sys
from contextlib import ExitStack

import numpy as np

sys.path.insert(0, "/opt/trn_rl_repo")

import concourse.bass as bass
import concourse.bacc as bacc
import concourse.tile as tile
from concourse import mybir
from concourse.bass_utils import run_bass_kernel_spmd

f32 = np.float32

B, L, H, C = 256, 512, 768, 9
M = 8                 # cores
NB = B // M           # 32 examples per core
NCH = 4               # L-chunks for the scans
CH = L // NCH         # 128
WARM = 16             # warmup steps for chunk entries
C2 = C * C            # 81
KCH = H // 128        # 6 K-chunks
TW = 16               # t-win per matmul chunk
NW = L // TW          # 32 matmul chunks
ENC = f32(16.0)       # tag encoding: enc = 16 - c

DT = mybir.dt.float32
DI = mybir.dt.int32
DU = mybir.dt.uint32

DEBUG = bool(int(os.environ.get("KERNEL_DEBUG", "0")))
TRACE = bool(int(os.environ.get("KERNEL_TRACE", "0")))

A = mybir.AluOpType
AF = mybir.ActivationFunctionType
AX = mybir.AxisListType


def _bc(ap, shape):
    return ap.to_broadcast(shape)


def build_program():
    nc = bacc.Bacc("TRN2", target_bir_lowering=False, debug=False)

    def din(name, shape, dt=DT):
        return nc.dram_tensor(name, list(shape), dt, kind="ExternalInput").ap()

    featsT = din("featsT", [H, L * NB])          # [h, w*512 + b*16 + t4]
    w_re = din("w_re", [128, KCH * C])
    bff = din("bff", [C, 1])
    tt81 = din("tt81", [128, C2])                # T[c,j] at j*9+c
    ett81 = din("ett81", [128, C2])              # exp(T)[c,j] at j*9+c
    tflat = din("tflat", [NB, C2])               # T[c,c'] at c*9+c'
    iota81 = din("iota81", [128, C2])            # c at (j*9+c)
    iotaenc = din("iotaenc", [128, C])           # 16 - c at col c
    expend = din("expend", [128, C])
    endr = din("endr", [128, C])
    startr = din("startr", [128, C])
    expstart = din("expstart", [128, C])
    ones32 = din("ones32", [NB, 1])
    onesN = din("onesN", [1, 512])
    ohL = din("ohL", [128, CH])
    ohLany = din("ohLany", [128, 1])
    mInv = din("mInv", [128, CH], DU)             # 1 where t=ch*128+i' is MASKED-OFF
    mAf = din("mAf", [128, CH])                   # mask (t<len) f32 at t=ch*128+i
    ch0m = din("ch0m", [128, 1], DU)
    rowm = [din(f"rowm{k}", [128, 1], DU) for k in range(NCH)]
    w1 = din("w1", [NB, C * L])
    counts = din("counts", [NB, C2])
    ohst = din("ohst", [NB, C])
    ohen = din("ohen", [NB, C])

    o_llh = nc.dram_tensor("llh_part", [1, 1], DT, kind="ExternalOutput").ap()
    o_dec = nc.dram_tensor("decoded", [NB, L], DI, kind="ExternalOutput").ap()
    dbg = {}
    if DEBUG:
        for n, s in dict(em_scan=[128, C * CH], G=[128, CH], LGc=[128, CH],
                         phist=[128, CH * C], D=[128, CH], logZ=[NB, NCH],
                         shist=[128, CH * C], bph=[128, CH * C],
                         traj=[128, CH * C], lastenc=[NB, 1], num=[NB, 1],
                         decf=[128, CH]).items():
            dbg[n] = nc.dram_tensor("dbg_" + n, s, DT, kind="ExternalOutput").ap()

    with tile.TileContext(nc) as tc, ExitStack() as ctx:
        pool = ctx.enter_context(tc.tile_pool(name="main", bufs=1))
        fpool = ctx.enter_context(tc.tile_pool(name="feats", bufs=3))
        ppool = ctx.enter_context(tc.tile_pool(name="ps", bufs=4, space="PSUM"))
        dpool = ctx.enter_context(tc.tile_pool(name="dsc", bufs=1, space="DRAM"))

        def const_tile(ap, shape, dt=DT, tag=None):
            t = pool.tile(shape, dt, tag=tag or ap.tensor.name)
            nc.sync.dma_start(t[:], ap)
            return t

        w_t = const_tile(w_re, [128, KCH * C])
        bff_t = const_tile(bff, [C, 1])
        tt81_t = const_tile(tt81, [128, C2])
        ett81_t = const_tile(ett81, [128, C2])
        tflat_t = const_tile(tflat, [NB, C2])
        io81_t = const_tile(iota81, [128, C2])
        ioenc_t = const_tile(iotaenc, [128, C])
        expend_t = const_tile(expend, [128, C])
        endr_t = const_tile(endr, [128, C])
        startr_t = const_tile(startr, [128, C])
        expstart_t = const_tile(expstart, [128, C])
        ones32_t = const_tile(ones32, [NB, 1])
        onesN_t = const_tile(onesN, [1, 512])
        ohL_t = const_tile(ohL, [128, CH])
        ohLany_t = const_tile(ohLany, [128, 1])
        mInv_t = const_tile(mInv, [128, CH], DU)
        mAf_t = const_tile(mAf, [128, CH])
        ch0m_t = const_tile(ch0m, [128, 1], DU)
        rowm_t = [const_tile(rowm[k], [128, 1], DU) for k in range(NCH)]
        w1_t = const_tile(w1, [NB, C * L])
        counts_t = const_tile(counts, [NB, C2])
        ohst_t = const_tile(ohst, [NB, C])
        ohen_t = const_tile(ohen, [NB, C])

        # ---- P1: em matmul ----
        # chunk (v, g): t-window v (128 t's), example group g (4 examples)
        # PSUM [c(9), (b4, t128)] -> d_gold[b, c*512+t] (dense 512B runs)
        d_gold = dpool.tile([NB, C * L], DT, tag="d_gold")     # b, c*512+t
        em_scan = pool.tile([128, C * CH], DT)
        em_gold = pool.tile([NB, C * L], DT)
        for v in range(NCH):
            for g in range(8):
                w = v * 8 + g
                ps = ppool.tile([C, 512], DT, tag="emps")
                for k in range(KCH):
                    fch = fpool.tile([128, 512], DT, tag="fch")
                    nc.sync.dma_start(fch[:], featsT[128 * k:128 * (k + 1),
                                                     512 * w:512 * (w + 1)])
                    nc.tensor.matmul(ps[:], w_t[:, C * k:C * (k + 1)], fch[:],
                                     start=(k == 0), stop=(k == KCH - 1))
                em_cb = fpool.tile([C, 512], DT, tag="em_cb")
                nc.scalar.activation(em_cb[:], ps[:], AF.Identity, bias=bff_t[:])
                # src iter (c, b4, t); dst d_gold[4g+b4, c*512 + 128v + t]
                nc.sync.dma_start(
                    d_gold[4 * g:4 * (g + 1), :]
                    .rearrange("b (c t) -> c b t", c=C)[:, :, 128 * v:128 * (v + 1)],
                    em_cb[:].rearrange("c (b t) -> c b t", b=4))
            # after each t-window: fill em_gold and em_scan pieces (dense)
            nc.sync.dma_start(
                em_gold[:].rearrange("b (c t) -> b c t", c=C)[:, :, 128 * v:128 * (v + 1)],
                d_gold[:].rearrange("b (c t) -> b c t", c=C)[:, :, 128 * v:128 * (v + 1)])
            nc.sync.dma_start(
                em_scan[32 * v:32 * (v + 1), :].rearrange("b (c i) -> b c i", c=C),
                d_gold[:].rearrange("b (c t) -> b c t", c=C)[:, :, 128 * v:128 * (v + 1)])
        if DEBUG:
            nc.sync.dma_start(dbg["em_scan"], em_scan[:])

        def ic(t, n=CH):   # [p, i, c] view of a [p, c*n] (c-outer) tile
            return t[:].rearrange("p (c i) -> p i c", c=C)

        # ---- P2: E, G, Etil, logG, LGc ----
        E_t = pool.tile([128, C * CH], DT)
        nc.scalar.activation(E_t[:], em_scan[:], AF.Exp)
        G_t = pool.tile([128, CH], DT)
        nc.vector.tensor_reduce(G_t[:], ic(E_t), AX.X, A.add)
        rG_t = pool.tile([128, CH], DT)
        nc.vector.reciprocal(rG_t[:], G_t[:])
        Etil = pool.tile([128, C * CH], DT)
        nc.vector.tensor_tensor(ic(Etil), ic(E_t),
                                rG_t[:].unsqueeze(2).to_broadcast((128, CH, C)),
                                A.mult)
        logG = pool.tile([128, CH], DT)
        nc.scalar.activation(logG[:], G_t[:], AF.Ln)
        zCH = pool.tile([128, CH], DT)
        nc.vector.memset(zCH[:], 0.0)
        LGc = pool.tile([128, CH], DT)
        nc.vector.tensor_tensor_scan(LGc[:], logG[:], zCH[:], 0.0, A.add, A.add)
        if DEBUG:
            nc.sync.dma_start(dbg["G"], G_t[:])
            nc.sync.dma_start(dbg["LGc"], LGc[:])

        # warmup slices from previous chunk (partition shift by -32)
        EtW = pool.tile([128, C * WARM], DT)
        nc.sync.dma_start(
            EtW[32:128, :].rearrange("p (c q) -> p c q", c=C),
            Etil[0:96, :].rearrange("p (c i) -> p c i", c=C)[:, :, CH - WARM:])
        nc.vector.memset(EtW[0:32, :], 1.0)

        # ---------- P3: forward exp-space scan ----------
        ph = pool.tile([128, CH * C], DT)          # free i*9+c
        st = pool.tile([128, C], DT)
        tmp81 = pool.tile([128, C2], DT, tag="tmp81")
        nc.vector.memset(st[:], 1.0 / C)

        def fwd_step(src_ap, dst_ap, etil_slice):
            nc.vector.tensor_tensor(
                tmp81[:].rearrange("p (j c) -> p j c", j=C),
                src_ap.unsqueeze(1).to_broadcast((128, C, C)),
                ett81_t[:].rearrange("p (j c) -> p j c", j=C), A.mult)
            v9 = pool.tile([128, C], DT, tag="v9f")
            nc.vector.tensor_reduce(v9[:], tmp81[:].rearrange("p (j c) -> p j c", j=C),
                                    AX.X, A.add)
            nc.vector.tensor_tensor(dst_ap, v9[:], etil_slice, A.mult)

        EtW_v = EtW[:].rearrange("p (c q) -> p q c", c=C)
        for q in range(WARM):
            fwd_step(st[:], st[:], EtW_v[:, q, :])
        ms = pool.tile([128, 1], DT)
        nc.vector.tensor_reduce(ms[:], st[:], AX.X, A.add)
        rms = pool.tile([128, 1], DT)
        nc.vector.reciprocal(rms[:], ms[:])
        nc.vector.tensor_scalar(st[:], st[:], rms[:], None, A.mult)

        Et_ic = ic(Etil)
        ph_ic = ph[:].rearrange("p (i c) -> p i c", i=CH)
        fwd_step(st[:], ph_ic[:, 0, :], Et_ic[:, 0, :])
        init0 = pool.tile([128, C], DT)
        nc.vector.tensor_tensor(init0[:], expstart_t[:], Et_ic[:, 0, :], A.mult)
        nc.vector.copy_predicated(ph_ic[:, 0, :], _bc(ch0m_t[:], (128, C)), init0[:])
        for i in range(1, CH):
            fwd_step(ph_ic[:, i - 1, :], ph_ic[:, i, :], Et_ic[:, i, :])
        if DEBUG:
            nc.sync.dma_start(dbg["phist"], ph[:])

        # ---------- P4: logZ ----------
        Dw = pool.tile([128, CH * C], DT, tag="Dw")
        nc.vector.tensor_tensor(Dw[:].rearrange("p (i c) -> p i c", i=CH), ph_ic,
                                expend_t[:].unsqueeze(1).to_broadcast((128, CH, C)),
                                A.mult)
        D_t = pool.tile([128, CH], DT)
        nc.vector.tensor_reduce(D_t[:], Dw[:].rearrange("p (i c) -> p i c", i=CH),
                                AX.X, A.add)
        logD = pool.tile([128, CH], DT)
        nc.scalar.activation(logD[:], D_t[:], AF.Ln)
        Gval = pool.tile([128, CH], DT)
        nc.vector.tensor_tensor(Gval[:], logD[:], LGc[:], A.add)
        zsel = pool.tile([128, CH], DT, tag="zsel")
        nc.vector.tensor_tensor(zsel[:], Gval[:], ohL_t[:], A.mult)
        zred = pool.tile([128, 1], DT)
        nc.vector.tensor_reduce(zred[:], zsel[:], AX.X, A.add)
        pm = pool.tile([128, 1], DT)
        nc.vector.tensor_reduce(pm[:], ph_ic[:, CH - 1, :], AX.X, A.add)
        lpm = pool.tile([128, 1], DT)
        nc.scalar.activation(lpm[:], pm[:], AF.Ln)
        LM = pool.tile([128, 1], DT)
        nc.vector.tensor_tensor(LM[:], lpm[:], LGc[:, CH - 1:CH], A.add)
        # regroup [128,1] -> [NB, NCH] via DRAM
        d_lm = dpool.tile([128, 1], DT, tag="d_lm")
        nc.sync.dma_start(d_lm[:], LM[:])
        LMb = pool.tile([NB, NCH], DT)
        nc.sync.dma_start(LMb[:], d_lm[:].rearrange("(k b) o -> b k o", k=NCH))
        LMi = pool.tile([NB, NCH], DT)
        zN = pool.tile([NB, NCH], DT, tag="zN")
        nc.vector.memset(zN[:], 0.0)
        nc.vector.tensor_tensor_scan(LMi[:], LMb[:], zN[:], 0.0, A.add, A.add)
        kap = pool.tile([NB, NCH], DT)
        nc.vector.tensor_tensor(kap[:], LMi[:], LMb[:], A.subtract)
        d_kap = dpool.tile([128, 1], DT, tag="d_kap")
        nc.sync.dma_start(d_kap[:].rearrange("(k b) o -> b k o", k=NCH), kap[:])
        kap128 = pool.tile([128, 1], DT)
        nc.sync.dma_start(kap128[:], d_kap[:])
        kapsel = pool.tile([128, 1], DT)
        nc.vector.tensor_tensor(kapsel[:], kap128[:], ohLany_t[:, 0:1], A.mult)
        zk = pool.tile([128, 1], DT)
        nc.vector.tensor_tensor(zk[:], zred[:], kapsel[:], A.add)
        d_zk = dpool.tile([128, 1], DT, tag="d_zk")
        nc.sync.dma_start(d_zk[:], zk[:])
        zkb = pool.tile([NB, NCH], DT)
        nc.sync.dma_start(zkb[:], d_zk[:].rearrange("(k b) o -> b k o", k=NCH))
        logZ = pool.tile([NB, 1], DT)
        nc.vector.tensor_reduce(logZ[:], zkb[:], AX.X, A.add)
        if DEBUG:
            nc.sync.dma_start(dbg["D"], D_t[:])
            nc.sync.dma_start(dbg["logZ"], zkb[:])

        # ---------- P5: viterbi forward scan (scores only, exact sequential) ----------
        # [NB partitions]; em slices come from em_gold[b, (c, t)] strided reads.
        shs = pool.tile([NB, L * C], DT)           # s-hist, free t*9+c
        shs_tc = shs[:].rearrange("b (t c) -> b t c", t=L)
        emg_ct = em_gold[:].rearrange("b (c t) -> b t c", c=C)
        tmp81b = pool.tile([NB, C2], DT, tag="tmp81b")
        nc.vector.tensor_tensor(shs_tc[:, 0, :], startr_t[0:NB, :], emg_ct[:, 0, :],
                                A.add)
        for t in range(1, L):
            nc.vector.tensor_tensor(
                tmp81b[:].rearrange("p (j c) -> p j c", j=C),
                shs_tc[:, t - 1, :].unsqueeze(1).to_broadcast((NB, C, C)),
                tt81_t[0:NB, :].rearrange("p (j c) -> p j c", j=C), A.add)
            mx = pool.tile([NB, C], DT, tag="mxv")
            nc.vector.tensor_reduce(mx[:], tmp81b[:].rearrange("p (j c) -> p j c", j=C),
                                    AX.X, A.max)
            nc.vector.tensor_tensor(shs_tc[:, t, :], mx[:], emg_ct[:, t, :], A.add)
        # rearrange s-hist to the (b,ch)-partition layout via DRAM
        d_sh = dpool.tile([128, CH * C], DT, tag="d_sh")
        nc.sync.dma_start(
            d_sh[:].rearrange("(k b) f -> b k f", k=NCH), shs[:])
        sh = pool.tile([128, CH * C], DT)
        nc.sync.dma_start(sh[:], d_sh[:])
        sh_ic = sh[:].rearrange("p (i c) -> p i c", i=CH)
        if DEBUG:
            nc.sync.dma_start(dbg["shist"], sh[:])

        # ---------- P6: batch bp extraction (enc space) ----------
        sprev0 = pool.tile([128, C], DT)
        nc.sync.dma_start(sprev0[32:128, :], sh[0:96, (CH - 1) * C:])
        nc.vector.memset(sprev0[0:32, :], 0.0)
        bph = pool.tile([128, CH * C], DT)         # bp_enc at free i*9+j
        BLK = 32

        def bp_block(sprev_ap, dst_ap, n):
            cb = pool.tile([128, BLK * C2], DT, tag="cb")
            cb4 = cb[:, 0:n * C2].rearrange("p (i j c) -> p i j c", j=C, c=C)
            nc.vector.tensor_tensor(
                cb4, sprev_ap.unsqueeze(2).to_broadcast((128, n, C, C)),
                tt81_t[:].rearrange("p (j c) -> p j c", j=C)
                .unsqueeze(1).to_broadcast((128, n, C, C)), A.add)
            mxb = pool.tile([128, BLK * C], DT, tag="mxb")
            mxb3 = mxb[:, 0:n * C].rearrange("p (i j) -> p i j", j=C)
            nc.vector.tensor_reduce(mxb3, cb4, AX.X, A.max)
            nc.vector.tensor_tensor(cb4, cb4,
                                    mxb3.unsqueeze(3).to_broadcast((128, n, C, C)),
                                    A.is_ge)
            eqb = pool.tile([128, BLK * C2], DT, tag="eqb")
            eqb4 = eqb[:, 0:n * C2].rearrange("p (i j c) -> p i j c", j=C, c=C)
            nc.vector.scalar_tensor_tensor(
                eqb4, cb4, float(ENC),
                io81_t[:].rearrange("p (j c) -> p j c", j=C)
                .unsqueeze(1).to_broadcast((128, n, C, C)),
                A.mult, A.subtract)
            nc.vector.tensor_reduce(dst_ap, eqb4, AX.X, A.max)

        bp_ij = bph[:].rearrange("p (i j) -> p i j", i=CH)
        bp_block(sprev0[:].unsqueeze(1), bp_ij[:, 0:1, :], 1)
        i = 1
        while i < CH:
            n = min(BLK, CH - i)
            bp_block(sh_ic[:, i - 1:i - 1 + n, :], bp_ij[:, i:i + n, :], n)
            i += n
        if DEBUG:
            nc.sync.dma_start(dbg["bph"], bph[:])

        bpx = pool.tile([128, C], DT)
        nc.sync.dma_start(bpx[0:96, :], bph[32:128, 0:C])
        nc.vector.memset(bpx[96:128, :], 0.0)

        # ---------- P7: backtrace phase A ----------
        traj = pool.tile([128, CH * C], DT)
        eq81 = pool.tile([128, C2], DT, tag="eq81")
        tr_ij = traj[:].rearrange("p (i j) -> p i j", i=CH)
        for ip in range(CH, 0, -1):
            bsl = bpx[:] if ip == CH else bp_ij[:, ip, :]
            mprev = ioenc_t[:] if ip == CH else tr_ij[:, ip, :]
            nc.vector.tensor_tensor(
                eq81[:].rearrange("p (j m) -> p j m", j=C),
                mprev.unsqueeze(2).to_broadcast((128, C, C)),
                ioenc_t[:].unsqueeze(1).to_broadcast((128, C, C)), A.is_equal)
            g81 = pool.tile([128, C2], DT, tag="g81")
            nc.vector.tensor_tensor(
                g81[:].rearrange("p (j m) -> p j m", j=C),
                eq81[:].rearrange("p (j m) -> p j m", j=C),
                bsl.unsqueeze(1).to_broadcast((128, C, C)), A.mult)
            dst = tr_ij[:, ip - 1, :]
            nc.vector.tensor_reduce(dst, g81[:].rearrange("p (j m) -> p j m", j=C),
                                    AX.X, A.add)
            nc.vector.copy_predicated(dst, _bc(mInv_t[:, ip - 1:ip], (128, C)),
                                      mprev)
        if DEBUG:
            nc.sync.dma_start(dbg["traj"], traj[:])

        # ---------- P8: last tag + entry chaining + decode ----------
        sselw = pool.tile([128, CH * C], DT, tag="sselw")
        nc.vector.tensor_tensor(sselw[:].rearrange("p (i c) -> p i c", i=CH), sh_ic,
                                ohL_t[:].unsqueeze(2).to_broadcast((128, CH, C)),
                                A.mult)
        ssel = pool.tile([128, C], DT)
        nc.vector.tensor_reduce(ssel[:],
                                sselw[:].rearrange("p (i c) -> p c i", i=CH),
                                AX.X, A.add)
        d_ss = dpool.tile([128, C], DT, tag="d_ss")
        nc.sync.dma_start(d_ss[:], ssel[:])
        sselb = pool.tile([NB, NCH * C], DT)
        nc.sync.dma_start(sselb[:], d_ss[:].rearrange("(k b) c -> b k c", k=NCH))
        sfin = pool.tile([NB, C], DT)
        nc.vector.tensor_reduce(sfin[:],
                                sselb[:].rearrange("b (k c) -> b c k", k=NCH),
                                AX.X, A.add)
        nc.vector.tensor_tensor(sfin[:], sfin[:], endr_t[0:NB, :], A.add)
        mxf = pool.tile([NB, 1], DT)
        nc.vector.tensor_reduce(mxf[:], sfin[:], AX.X, A.max)
        eqf = pool.tile([NB, C], DT)
        nc.vector.tensor_scalar(eqf[:], sfin[:], mxf[:], None, A.is_ge)
        vf = pool.tile([NB, C], DT)
        nc.vector.scalar_tensor_tensor(vf[:], eqf[:], float(ENC), io81_t[0:NB, 0:C],
                                       A.mult, A.subtract)
        lastenc = pool.tile([NB, 1], DT)
        nc.vector.tensor_reduce(lastenc[:], vf[:], AX.X, A.max)
        if DEBUG:
            nc.sync.dma_start(dbg["lastenc"], lastenc[:])

        ent128 = pool.tile([128, 1], DT)
        nc.vector.memset(ent128[:], 0.0)
        decf = pool.tile([128, CH], DT)
        entb = pool.tile([NB, 1], DT, tag="entb")
        nc.vector.tensor_copy(entb[:], lastenc[:])
        for k in range(NCH - 1, -1, -1):
            nc.sync.dma_start(ent128[32 * k:32 * (k + 1), :], entb[:])
            ohe = pool.tile([128, C], DT, tag="ohe")
            nc.vector.tensor_scalar(ohe[:], ioenc_t[:], ent128[:], None, A.is_equal)
            selw = pool.tile([128, CH * C], DT, tag="selw")
            nc.vector.tensor_tensor(
                selw[:].rearrange("p (i j) -> p i j", i=CH), tr_ij,
                ohe[:].unsqueeze(1).to_broadcast((128, CH, C)), A.mult)
            dsel = pool.tile([128, CH], DT, tag="dsel")
            nc.vector.tensor_reduce(dsel[:],
                                    selw[:].rearrange("p (i j) -> p i j", i=CH),
                                    AX.X, A.add)
            nc.vector.copy_predicated(decf[:], _bc(rowm_t[k][:], (128, CH)), dsel[:])
            nc.sync.dma_start(entb[:], dsel[32 * k:32 * (k + 1), 0:1])
        nc.vector.tensor_scalar(decf[:], decf[:], float(-ENC), -1.0, A.add, op1=A.mult)
        nc.vector.tensor_tensor(decf[:], decf[:], mAf_t[:], A.mult)
        if DEBUG:
            nc.sync.dma_start(dbg["decf"], decf[:])
        deci = pool.tile([128, CH], DI)
        nc.vector.tensor_copy(deci[:], decf[:])
        for k in range(NCH):
            nc.sync.dma_start(o_dec[:, CH * k:CH * (k + 1)],
                              deci[32 * k:32 * (k + 1), :])

        # ---------- P9: gold numerator + llh partial ----------
        junk = pool.tile([NB, C * L], DT, tag="junk")
        emp = pool.tile([NB, 1], DT)
        nc.vector.scalar_tensor_tensor(junk[:], em_gold[:], 1.0, w1_t[:],
                                       A.mult, A.mult, accum_out=emp[:])
        tw_ = pool.tile([NB, C2], DT, tag="tw_")
        nc.vector.tensor_tensor(tw_[:], counts_t[:], tflat_t[:], A.mult)
        tp = pool.tile([NB, 1], DT)
        nc.vector.tensor_reduce(tp[:], tw_[:], AX.X, A.add)
        sw_ = pool.tile([NB, C], DT, tag="sw_")
        nc.vector.tensor_tensor(sw_[:], ohst_t[:], startr_t[0:NB, :], A.mult)
        sp = pool.tile([NB, 1], DT)
        nc.vector.tensor_reduce(sp[:], sw_[:], AX.X, A.add)
        ew_ = pool.tile([NB, C], DT, tag="ew_")
        nc.vector.tensor_tensor(ew_[:], ohen_t[:], endr_t[0:NB, :], A.mult)
        ep = pool.tile([NB, 1], DT)
        nc.vector.tensor_reduce(ep[:], ew_[:], AX.X, A.add)
        num = pool.tile([NB, 1], DT)
        nc.vector.tensor_tensor(num[:], emp[:], tp[:], A.add)
        nc.vector.tensor_tensor(num[:], num[:], sp[:], A.add)
        nc.vector.tensor_tensor(num[:], num[:], ep[:], A.add)
        if DEBUG:
            nc.sync.dma_start(dbg["num"], num[:])
        diff = pool.tile([NB, 1], DT)
        nc.vector.tensor_tensor(diff[:], num[:], logZ[:], A.subtract)
        acc_ps = ppool.tile([1, 1], DT, tag="accps")
        nc.tensor.matmul(acc_ps[:], diff[:], ones32_t[:], start=True, stop=True)
        acc = pool.tile([1, 1], DT)
        nc.scalar.copy(acc[:], acc_ps[:])
        nc.sync.dma_start(o_llh, acc[:])

    nc.compile()
    return nc


def host_prep(inputs):
    feats = np.asarray(inputs["feats"], f32)
    W = np.asarray(inputs["W_ff"], f32)
    b_ff = np.asarray(inputs["b_ff"], f32)
    start = np.asarray(inputs["start_transitions"], f32)
    end = np.asarray(inputs["end_transitions"], f32)
    T = np.asarray(inputs["transitions"], f32)
    tags = np.asarray(inputs["tags"])
    lengths = np.asarray(inputs["lengths"])

    expT = np.exp(T).astype(f32)
    rep = lambda a: np.ascontiguousarray(
        np.tile(np.asarray(a, f32).reshape(1, -1), (128, 1)))
    tt81 = rep(T.T.reshape(-1))
    ett81 = rep(expT.T.reshape(-1))
    iota81 = rep(np.tile(np.arange(C, dtype=f32), C))
    iotaenc = rep(ENC - np.arange(C, dtype=f32))
    expend = rep(np.exp(end))
    endr = rep(end)
    startr = rep(start)
    expstart = rep(np.exp(start))
    w_re = np.ascontiguousarray(
        W.reshape(KCH, 128, C).transpose(1, 0, 2).reshape(128, KCH * C)).astype(f32)

    shared = dict(
        w_re=w_re, bff=np.ascontiguousarray(b_ff.reshape(C, 1)).astype(f32),
        tt81=tt81, ett81=ett81, iota81=iota81, iotaenc=iotaenc,
        expend=expend, endr=endr, startr=startr, expstart=expstart,
        tflat=np.ascontiguousarray(np.tile(T.reshape(1, -1), (NB, 1))).astype(f32),
        ones32=np.ones((NB, 1), f32), onesN=np.ones((1, 512), f32),
    )

    in_maps = []
    bidx = np.arange(NB)
    for c in range(M):
        sl = slice(c * NB, (c + 1) * NB)
        fe = feats[sl]
        tg = tags[sl].astype(np.int64)
        ln = lengths[sl].astype(np.int64)
        # chunk (v,g): featsT[h, 512*(8v+g) + 128*b4 + t'] = fe[4g+b4, 128v+t', h]
        fw = fe.reshape(8, 4, NCH, CH, H).transpose(4, 2, 0, 1, 3).reshape(H, L * NB)
        featsT = np.ascontiguousarray(fw).astype(f32)

        m = (np.arange(L)[None, :] < ln[:, None])
        ohL = np.zeros((128, CH), f32)
        ohLany = np.zeros((128, 1), f32)
        mInv = np.zeros((128, CH), np.uint32)
        mAf = np.zeros((128, CH), f32)
        ch0m = np.zeros((128, 1), np.uint32)
        ch0m[0:32] = 1
        rowms = []
        for ch in range(NCH):
            t0 = ch * CH
            for b in range(NB):
                p = 32 * ch + b
                lm1 = int(ln[b]) - 1
                if t0 <= lm1 < t0 + CH:
                    ohL[p, lm1 - t0] = 1.0
                    ohLany[p, 0] = 1.0
                tv = np.arange(t0 + 1, t0 + CH + 1)
                mInv[p, :] = (tv >= ln[b]).astype(np.uint32)
                mAf[p, :] = m[b, t0:t0 + CH].astype(f32)
            rm = np.zeros((128, 1), np.uint32)
            rm[32 * ch:32 * (ch + 1)] = 1
            rowms.append(rm)
        w1 = np.zeros((NB, C, L), f32)
        for t in range(L):
            w1[bidx, tg[:, t], t] = m[:, t].astype(f32)
        counts = np.zeros((NB, C2), f32)
        for b in range(NB):
            for t in range(1, int(ln[b])):
                counts[b, tg[b, t - 1] * C + tg[b, t]] += 1
        ohst = np.zeros((NB, C), f32)
        ohst[bidx, tg[:, 0]] = 1
        ohen = np.zeros((NB, C), f32)
        ohen[bidx, tg[bidx, ln - 1]] = 1

        im = dict(shared)
        im.update(featsT=featsT, ohL=ohL, ohLany=ohLany, mInv=mInv, mAf=mAf,
                  ch0m=ch0m, w1=np.ascontiguousarray(w1.reshape(NB, C * L)),
                  counts=counts, ohst=ohst, ohen=ohen)
        for k in range(NCH):
            im[f"rowm{k}"] = rowms[k]
        in_maps.append(im)
    return in_maps


_prog_cache = {}


def get_program():
    if "nc" not in _prog_cache:
        _prog_cache["nc"] = build_program()
    return _prog_cache["nc"]


def _install_ntff_hook():
    """Provide antenv.axon_hooks via ctypes on images that lack it."""
    import types
    import ctypes
    import contextlib
    try:
        from antenv.axon_hooks import get_axon_ntff_profile_hook  # noqa: F401
        return
    except ImportError:
        pass
    try:
        lib = ctypes.CDLL("/opt/axon/libaxon_pjrt.so")
        if not hasattr(lib, "axon_start_nrt_profile"):
            return
    except OSError:
        return
    lib.axon_start_nrt_profile.argtypes = [ctypes.POINTER(ctypes.c_int64),
                                           ctypes.c_size_t]
    lib.axon_start_nrt_profile.restype = ctypes.c_int64
    lib.axon_stop_nrt_profile.argtypes = [ctypes.c_char_p]
    lib.axon_stop_nrt_profile.restype = ctypes.c_int64

    @contextlib.contextmanager
    def _hook(output_dir, device_ids):
        import jax
        jax.devices()
        if device_ids:
            ids = (ctypes.c_int64 * len(device_ids))(*device_ids)
            rc = lib.axon_start_nrt_profile(ids, len(device_ids))
        else:
            rc = lib.axon_start_nrt_profile(None, 0)
        if rc != 0:
            raise RuntimeError(f"axon_start_nrt_profile rc={rc}")
        try:
            yield
        finally:
            n = lib.axon_stop_nrt_profile(str(output_dir).encode())
            print(f"ntff profile: {n} file(s) -> {output_dir}")

    mod = types.ModuleType("antenv.axon_hooks")
    mod.get_axon_ntff_profile_hook = lambda: _hook
    mod.set_axon_ntff_profile_hook = lambda h: None
    sys.modules["antenv.axon_hooks"] = mod


def kernel(**inputs):
    nc = get_program()
    in_maps = host_prep(inputs)
    if TRACE:
        _install_ntff_hook()
    res = run_bass_kernel_spmd(nc, in_maps, list(range(M)), trace=TRACE)
    llh = np.sum([r["llh_part"][0, 0] for r in res.results], dtype=f32) / f32(B)
    decoded = np.concatenate([r["decoded"] for r in res.results], 0).astype(np.int32)
    kernel.last_results = res
    kernel.last_exec_time_ns = getattr(res, "exec_time_ns", None)
    return np.float32(llh), decoded


# revision 16
# speedup vs baseline: 1.5268x; 1.1406x over previous
"""Trainium2 Bass kernel for the EntityBert CRF loss (llh + viterbi decode).

kernel(**inputs) takes FULL inputs (B=256,L=512,H=768,C=9), shards batch
across 8 NeuronCores, runs one SPMD Bass/Tile program per core, returns
(llh_scalar_f32, decoded_int32[B,L]) matching reference semantics.

Algorithm per core (NB=32 examples):
  em = feats @ W + b  (PE, fp32, t-windowed so scans can pipeline)
  forward logsumexp scan in exp space, G-normalized, chunked over L in 4
    chunks on (b,chunk)=128 partitions with provably-contracting warmup;
    per-chunk mass chained; logZ gathered at len-1 via host one-hots.
  viterbi forward scan (scores only) same chunking; backpointers extracted
    in a batch pass; backtrace via exact map-composition over chunks;
    decode selected by entry tags.
  gold-path numerator via host-precomputed int-derived one-hot weights.
Host does sharding + int-derived constant prep only; all float math on device.
"""
import os
import sys
from contextlib import ExitStack

import numpy as np

sys.path.insert(0, "/opt/trn_rl_repo")

import concourse.bass as bass
import concourse.bacc as bacc
import concourse.tile as tile
from concourse import mybir
from concourse.bass_utils import run_bass_kernel_spmd

f32 = np.float32

B, L, H, C = 256, 512, 768, 9
M = 8                 # cores
NB = B // M           # 32 examples per core
NCH = 4               # L-chunks for the scans
CH = L // NCH         # 128
WARM = 16             # warmup steps for chunk entries
C2 = C * C            # 81
KCH = H // 128        # 6 K-chunks
TW = 16               # t-win per matmul chunk
NW = L // TW          # 32 matmul chunks
ENC = f32(16.0)       # tag encoding: enc = 16 - c

DT = mybir.dt.float32
DI = mybir.dt.int32
DU = mybir.dt.uint32

DEBUG = bool(int(os.environ.get("KERNEL_DEBUG", "0")))
TRACE = bool(int(os.environ.get("KERNEL_TRACE", "0")))

A = mybir.AluOpType
AF = mybir.ActivationFunctionType
AX = mybir.AxisListType


def _bc(ap, shape):
    return ap.to_broadcast(shape)


def build_program():
    nc = bacc.Bacc("TRN2", target_bir_lowering=False, debug=False)

    def din(name, shape, dt=DT):
        return nc.dram_tensor(name, list(shape), dt, kind="ExternalInput").ap()

    featsT = din("featsT", [H, L * NB])          # [h, w*512 + b*16 + t4]
    w_re = din("w_re", [128, KCH * C])
    bff = din("bff", [C, 1])
    tt81 = din("tt81", [128, C2])                # T[c,j] at j*9+c
    ett81 = din("ett81", [128, C2])              # exp(T)[c,j] at j*9+c
    tflat = din("tflat", [NB, C2])               # T[c,c'] at c*9+c'
    iota81 = din("iota81", [128, C2])            # c at (j*9+c)
    iotaenc = din("iotaenc", [128, C])           # 16 - c at col c
    expend = din("expend", [128, C])
    endr = din("endr", [128, C])
    startr = din("startr", [128, C])
    expstart = din("expstart", [128, C])
    ones32 = din("ones32", [NB, 1])
    onesN = din("onesN", [1, 512])
    ohL = din("ohL", [128, CH])
    ohLany = din("ohLany", [128, 1])
    mInv = din("mInv", [128, CH], DU)             # 1 where t=ch*128+i' is MASKED-OFF
    mAf = din("mAf", [128, CH])                   # mask (t<len) f32 at t=ch*128+i
    ch0m = din("ch0m", [128, 1], DU)
    rowm = [din(f"rowm{k}", [128, 1], DU) for k in range(NCH)]
    w1 = din("w1", [NB, C * L])
    counts = din("counts", [NB, C2])
    ohst = din("ohst", [NB, C])
    ohen = din("ohen", [NB, C])

    o_llh = nc.dram_tensor("llh_part", [1, 1], DT, kind="ExternalOutput").ap()
    o_dec = nc.dram_tensor("decoded", [NB, L], DI, kind="ExternalOutput").ap()
    dbg = {}
    if DEBUG:
        for n, s in dict(em_scan=[128, C * CH], G=[128, CH], LGc=[128, CH],
                         phist=[128, CH * C], D=[128, CH], logZ=[NB, NCH],
                         shist=[128, CH * C], bph=[128, CH * C],
                         traj=[128, CH * C], lastenc=[NB, 1], num=[NB, 1],
                         decf=[128, CH]).items():
            dbg[n] = nc.dram_tensor("dbg_" + n, s, DT, kind="ExternalOutput").ap()

    with tile.TileContext(nc) as tc, ExitStack() as ctx:
        pool = ctx.enter_context(tc.tile_pool(name="main", bufs=1))
        fpool = ctx.enter_context(tc.tile_pool(name="feats", bufs=12))
        ppool = ctx.enter_context(tc.tile_pool(name="ps", bufs=7, space="PSUM"))
        ppool2 = ctx.enter_context(tc.tile_pool(name="ps2", bufs=1, space="PSUM"))
        dpool = ctx.enter_context(tc.tile_pool(name="dsc", bufs=1, space="DRAM"))

        def const_tile(ap, shape, dt=DT, tag=None):
            t = pool.tile(shape, dt, tag=tag or ap.tensor.name)
            nc.sync.dma_start(t[:], ap)
            return t

        w_t = const_tile(w_re, [128, KCH * C])
        bff_t = const_tile(bff, [C, 1])
        tt81_t = const_tile(tt81, [128, C2])
        ett81_t = const_tile(ett81, [128, C2])
        tflat_t = const_tile(tflat, [NB, C2])
        io81_t = const_tile(iota81, [128, C2])
        ioenc_t = const_tile(iotaenc, [128, C])
        expend_t = const_tile(expend, [128, C])
        endr_t = const_tile(endr, [128, C])
        startr_t = const_tile(startr, [128, C])
        expstart_t = const_tile(expstart, [128, C])
        ones32_t = const_tile(ones32, [NB, 1])
        onesN_t = const_tile(onesN, [1, 512])
        ohL_t = const_tile(ohL, [128, CH])
        ohLany_t = const_tile(ohLany, [128, 1])
        mInv_t = const_tile(mInv, [128, CH], DU)
        mAf_t = const_tile(mAf, [128, CH])
        ch0m_t = const_tile(ch0m, [128, 1], DU)
        rowm_t = [const_tile(rowm[k], [128, 1], DU) for k in range(NCH)]
        w1_t = const_tile(w1, [NB, C * L])
        counts_t = const_tile(counts, [NB, C2])
        ohst_t = const_tile(ohst, [NB, C])
        ohen_t = const_tile(ohen, [NB, C])

        # ---- P1: em matmul ----
        # chunk (v, g): t-window v (128 t's), example group g (4 examples)
        # PSUM [c(9), (b4, t128)] -> d_gold[b, c*512+t] (dense 512B runs)
        d_gold = dpool.tile([NB, C * L], DT, tag="d_gold")     # b, c*512+t
        em_scan = pool.tile([128, C * CH], DT)
        em_gold = pool.tile([NB, C * L], DT)
        for v in range(NCH):
            for g in range(8):
                w = v * 8 + g
                ps = ppool.tile([C, 512], DT, tag="emps")
                for k in range(KCH):
                    fch = fpool.tile([128, 512], DT, tag="fch")
                    eng = nc.sync if (w * KCH + k) % 2 == 0 else nc.scalar
                    eng.dma_start(fch[:], featsT[128 * k:128 * (k + 1),
                                                 512 * w:512 * (w + 1)])
                    nc.tensor.matmul(ps[:], w_t[:, C * k:C * (k + 1)], fch[:],
                                     start=(k == 0), stop=(k == KCH - 1))
                em_cb = fpool.tile([C, 512], DT, tag="em_cb")
                nc.scalar.activation(em_cb[:], ps[:], AF.Identity, bias=bff_t[:])
                # src iter (c, b4, t); dst d_gold[4g+b4, c*512 + 128v + t]
                nc.sync.dma_start(
                    d_gold[4 * g:4 * (g + 1), :]
                    .rearrange("b (c t) -> c b t", c=C)[:, :, 128 * v:128 * (v + 1)],
                    em_cb[:].rearrange("c (b t) -> c b t", b=4))
            # after each t-window: fill em_gold and em_scan pieces (dense)
            nc.sync.dma_start(
                em_gold[:].rearrange("b (c t) -> b c t", c=C)[:, :, 128 * v:128 * (v + 1)],
                d_gold[:].rearrange("b (c t) -> b c t", c=C)[:, :, 128 * v:128 * (v + 1)])
            nc.sync.dma_start(
                em_scan[32 * v:32 * (v + 1), :].rearrange("b (c i) -> b c i", c=C),
                d_gold[:].rearrange("b (c t) -> b c t", c=C)[:, :, 128 * v:128 * (v + 1)])
        if DEBUG:
            nc.sync.dma_start(dbg["em_scan"], em_scan[:])

        def ic(t, n=CH):   # [p, i, c] view of a [p, c*n] (c-outer) tile
            return t[:].rearrange("p (c i) -> p i c", c=C)

        # ---- P2: E, G, Etil, logG, LGc ----
        E_t = pool.tile([128, C * CH], DT)
        nc.scalar.activation(E_t[:], em_scan[:], AF.Exp)
        G_t = pool.tile([128, CH], DT)
        nc.vector.tensor_reduce(G_t[:], ic(E_t), AX.X, A.add)
        rG_t = pool.tile([128, CH], DT)
        nc.vector.reciprocal(rG_t[:], G_t[:])
        Etil = pool.tile([128, C * CH], DT)
        nc.vector.tensor_tensor(ic(Etil), ic(E_t),
                                rG_t[:].unsqueeze(2).to_broadcast((128, CH, C)),
                                A.mult)
        logG = pool.tile([128, CH], DT)
        nc.scalar.activation(logG[:], G_t[:], AF.Ln)
        zCH = pool.tile([128, CH], DT)
        nc.vector.memset(zCH[:], 0.0)
        LGc = pool.tile([128, CH], DT)
        nc.vector.tensor_tensor_scan(LGc[:], logG[:], zCH[:], 0.0, A.add, A.add)
        if DEBUG:
            nc.sync.dma_start(dbg["G"], G_t[:])
            nc.sync.dma_start(dbg["LGc"], LGc[:])

        # warmup slices from previous chunk (partition shift by -32)
        EtW = pool.tile([128, C * WARM], DT)
        nc.sync.dma_start(
            EtW[32:128, :].rearrange("p (c q) -> p c q", c=C),
            Etil[0:96, :].rearrange("p (c i) -> p c i", c=C)[:, :, CH - WARM:])
        nc.vector.memset(EtW[0:32, :], 1.0)

        # ---------- P3: forward exp-space scan ----------
        ph = pool.tile([128, CH * C], DT)          # free i*9+c
        st = pool.tile([128, C], DT)
        tmp81 = pool.tile([128, C2], DT, tag="tmp81")
        nc.vector.memset(st[:], 1.0 / C)

        def fwd_step(src_ap, dst_ap, etil_slice):
            nc.vector.tensor_tensor(
                tmp81[:].rearrange("p (j c) -> p j c", j=C),
                src_ap.unsqueeze(1).to_broadcast((128, C, C)),
                ett81_t[:].rearrange("p (j c) -> p j c", j=C), A.mult)
            v9 = pool.tile([128, C], DT, tag="v9f")
            nc.vector.tensor_reduce(v9[:], tmp81[:].rearrange("p (j c) -> p j c", j=C),
                                    AX.X, A.add)
            nc.vector.tensor_tensor(dst_ap, v9[:], etil_slice, A.mult)

        EtW_v = EtW[:].rearrange("p (c q) -> p q c", c=C)
        for q in range(WARM):
            fwd_step(st[:], st[:], EtW_v[:, q, :])
        ms = pool.tile([128, 1], DT)
        nc.vector.tensor_reduce(ms[:], st[:], AX.X, A.add)
        rms = pool.tile([128, 1], DT)
        nc.vector.reciprocal(rms[:], ms[:])
        nc.vector.tensor_scalar(st[:], st[:], rms[:], None, A.mult)

        Et_ic = ic(Etil)
        ph_ic = ph[:].rearrange("p (i c) -> p i c", i=CH)
        fwd_step(st[:], ph_ic[:, 0, :], Et_ic[:, 0, :])
        init0 = pool.tile([128, C], DT)
        nc.vector.tensor_tensor(init0[:], expstart_t[:], Et_ic[:, 0, :], A.mult)
        nc.vector.copy_predicated(ph_ic[:, 0, :], _bc(ch0m_t[:], (128, C)), init0[:])
        for i in range(1, CH):
            fwd_step(ph_ic[:, i - 1, :], ph_ic[:, i, :], Et_ic[:, i, :])
        if DEBUG:
            nc.sync.dma_start(dbg["phist"], ph[:])

        # ---------- P4: logZ ----------
        Dw = pool.tile([128, CH * C], DT, tag="Dw")
        nc.vector.tensor_tensor(Dw[:].rearrange("p (i c) -> p i c", i=CH), ph_ic,
                                expend_t[:].unsqueeze(1).to_broadcast((128, CH, C)),
                                A.mult)
        D_t = pool.tile([128, CH], DT)
        nc.vector.tensor_reduce(D_t[:], Dw[:].rearrange("p (i c) -> p i c", i=CH),
                                AX.X, A.add)
        logD = pool.tile([128, CH], DT)
        nc.scalar.activation(logD[:], D_t[:], AF.Ln)
        Gval = pool.tile([128, CH], DT)
        nc.vector.tensor_tensor(Gval[:], logD[:], LGc[:], A.add)
        zsel = pool.tile([128, CH], DT, tag="zsel")
        nc.vector.tensor_tensor(zsel[:], Gval[:], ohL_t[:], A.mult)
        zred = pool.tile([128, 1], DT)
        nc.vector.tensor_reduce(zred[:], zsel[:], AX.X, A.add)
        pm = pool.tile([128, 1], DT)
        nc.vector.tensor_reduce(pm[:], ph_ic[:, CH - 1, :], AX.X, A.add)
        lpm = pool.tile([128, 1], DT)
        nc.scalar.activation(lpm[:], pm[:], AF.Ln)
        LM = pool.tile([128, 1], DT)
        nc.vector.tensor_tensor(LM[:], lpm[:], LGc[:, CH - 1:CH], A.add)
        # regroup [128,1] -> [NB, NCH] via DRAM
        d_lm = dpool.tile([128, 1], DT, tag="d_lm")
        nc.sync.dma_start(d_lm[:], LM[:])
        LMb = pool.tile([NB, NCH], DT)
        nc.sync.dma_start(LMb[:], d_lm[:].rearrange("(k b) o -> b k o", k=NCH))
        LMi = pool.tile([NB, NCH], DT)
        zN = pool.tile([NB, NCH], DT, tag="zN")
        nc.vector.memset(zN[:], 0.0)
        nc.vector.tensor_tensor_scan(LMi[:], LMb[:], zN[:], 0.0, A.add, A.add)
        kap = pool.tile([NB, NCH], DT)
        nc.vector.tensor_tensor(kap[:], LMi[:], LMb[:], A.subtract)
        d_kap = dpool.tile([128, 1], DT, tag="d_kap")
        nc.sync.dma_start(d_kap[:].rearrange("(k b) o -> b k o", k=NCH), kap[:])
        kap128 = pool.tile([128, 1], DT)
        nc.sync.dma_start(kap128[:], d_kap[:])
        kapsel = pool.tile([128, 1], DT)
        nc.vector.tensor_tensor(kapsel[:], kap128[:], ohLany_t[:, 0:1], A.mult)
        zk = pool.tile([128, 1], DT)
        nc.vector.tensor_tensor(zk[:], zred[:], kapsel[:], A.add)
        d_zk = dpool.tile([128, 1], DT, tag="d_zk")
        nc.sync.dma_start(d_zk[:], zk[:])
        zkb = pool.tile([NB, NCH], DT)
        nc.sync.dma_start(zkb[:], d_zk[:].rearrange("(k b) o -> b k o", k=NCH))
        logZ = pool.tile([NB, 1], DT)
        nc.vector.tensor_reduce(logZ[:], zkb[:], AX.X, A.add)
        if DEBUG:
            nc.sync.dma_start(dbg["D"], D_t[:])
            nc.sync.dma_start(dbg["logZ"], zkb[:])

        # ---------- P5: viterbi forward scan (scores only, exact sequential) ----------
        # [NB partitions]; em slices come from em_gold[b, (c, t)] strided reads.
        shs = pool.tile([NB, L * C], DT)           # s-hist, free t*9+c
        shs_tc = shs[:].rearrange("b (t c) -> b t c", t=L)
        emg_ct = em_gold[:].rearrange("b (c t) -> b t c", c=C)
        tmp81b = pool.tile([NB, C2], DT, tag="tmp81b")
        nc.vector.tensor_tensor(shs_tc[:, 0, :], startr_t[0:NB, :], emg_ct[:, 0, :],
                                A.add)
        for t in range(1, L):
            nc.vector.tensor_tensor(
                tmp81b[:].rearrange("p (j c) -> p j c", j=C),
                shs_tc[:, t - 1, :].unsqueeze(1).to_broadcast((NB, C, C)),
                tt81_t[0:NB, :].rearrange("p (j c) -> p j c", j=C), A.add)
            mx = pool.tile([NB, C], DT, tag="mxv")
            nc.vector.tensor_reduce(mx[:], tmp81b[:].rearrange("p (j c) -> p j c", j=C),
                                    AX.X, A.max)
            nc.vector.tensor_tensor(shs_tc[:, t, :], mx[:], emg_ct[:, t, :], A.add)
        # rearrange s-hist to the (b,ch)-partition layout via DRAM
        d_sh = dpool.tile([128, CH * C], DT, tag="d_sh")
        nc.sync.dma_start(
            d_sh[:].rearrange("(k b) f -> b k f", k=NCH), shs[:])
        sh = pool.tile([128, CH * C], DT)
        nc.sync.dma_start(sh[:], d_sh[:])
        sh_ic = sh[:].rearrange("p (i c) -> p i c", i=CH)
        if DEBUG:
            nc.sync.dma_start(dbg["shist"], sh[:])

        # ---------- P6: batch bp extraction (enc space) ----------
        sprev0 = pool.tile([128, C], DT)
        nc.sync.dma_start(sprev0[32:128, :], sh[0:96, (CH - 1) * C:])
        nc.vector.memset(sprev0[0:32, :], 0.0)
        bph = pool.tile([128, CH * C], DT)         # bp_enc at free i*9+j
        BLK = 32

        def bp_block(sprev_ap, dst_ap, n):
            cb = pool.tile([128, BLK * C2], DT, tag="cb")
            cb4 = cb[:, 0:n * C2].rearrange("p (i j c) -> p i j c", j=C, c=C)
            nc.vector.tensor_tensor(
                cb4, sprev_ap.unsqueeze(2).to_broadcast((128, n, C, C)),
                tt81_t[:].rearrange("p (j c) -> p j c", j=C)
                .unsqueeze(1).to_broadcast((128, n, C, C)), A.add)
            mxb = pool.tile([128, BLK * C], DT, tag="mxb")
            mxb3 = mxb[:, 0:n * C].rearrange("p (i j) -> p i j", j=C)
            nc.vector.tensor_reduce(mxb3, cb4, AX.X, A.max)
            nc.vector.tensor_tensor(cb4, cb4,
                                    mxb3.unsqueeze(3).to_broadcast((128, n, C, C)),
                                    A.is_ge)
            eqb = pool.tile([128, BLK * C2], DT, tag="eqb")
            eqb4 = eqb[:, 0:n * C2].rearrange("p (i j c) -> p i j c", j=C, c=C)
            nc.vector.scalar_tensor_tensor(
                eqb4, cb4, float(ENC),
                io81_t[:].rearrange("p (j c) -> p j c", j=C)
                .unsqueeze(1).to_broadcast((128, n, C, C)),
                A.mult, A.subtract)
            nc.vector.tensor_reduce(dst_ap, eqb4, AX.X, A.max)

        bp_ij = bph[:].rearrange("p (i j) -> p i j", i=CH)
        bp_block(sprev0[:].unsqueeze(1), bp_ij[:, 0:1, :], 1)
        i = 1
        while i < CH:
            n = min(BLK, CH - i)
            bp_block(sh_ic[:, i - 1:i - 1 + n, :], bp_ij[:, i:i + n, :], n)
            i += n
        if DEBUG:
            nc.sync.dma_start(dbg["bph"], bph[:])

        bpx = pool.tile([128, C], DT)
        nc.sync.dma_start(bpx[0:96, :], bph[32:128, 0:C])
        nc.vector.memset(bpx[96:128, :], 0.0)

        # ---------- P7: backtrace phase A ----------
        traj = pool.tile([128, CH * C], DT)
        eq81 = pool.tile([128, C2], DT, tag="eq81")
        tr_ij = traj[:].rearrange("p (i j) -> p i j", i=CH)
        for ip in range(CH, 0, -1):
            bsl = bpx[:] if ip == CH else bp_ij[:, ip, :]
            mprev = ioenc_t[:] if ip == CH else tr_ij[:, ip, :]
            nc.vector.tensor_tensor(
                eq81[:].rearrange("p (j m) -> p j m", j=C),
                mprev.unsqueeze(2).to_broadcast((128, C, C)),
                ioenc_t[:].unsqueeze(1).to_broadcast((128, C, C)), A.is_equal)
            g81 = pool.tile([128, C2], DT, tag="g81")
            nc.vector.tensor_tensor(
                g81[:].rearrange("p (j m) -> p j m", j=C),
                eq81[:].rearrange("p (j m) -> p j m", j=C),
                bsl.unsqueeze(1).to_broadcast((128, C, C)), A.mult)
            dst = tr_ij[:, ip - 1, :]
            nc.vector.tensor_reduce(dst, g81[:].rearrange("p (j m) -> p j m", j=C),
                                    AX.X, A.add)
            nc.vector.copy_predicated(dst, _bc(mInv_t[:, ip - 1:ip], (128, C)),
                                      mprev)
        if DEBUG:
            nc.sync.dma_start(dbg["traj"], traj[:])

        # ---------- P8: last tag + entry chaining + decode ----------
        sselw = pool.tile([128, CH * C], DT, tag="sselw")
        nc.vector.tensor_tensor(sselw[:].rearrange("p (i c) -> p i c", i=CH), sh_ic,
                                ohL_t[:].unsqueeze(2).to_broadcast((128, CH, C)),
                                A.mult)
        ssel = pool.tile([128, C], DT)
        nc.vector.tensor_reduce(ssel[:],
                                sselw[:].rearrange("p (i c) -> p c i", i=CH),
                                AX.X, A.add)
        d_ss = dpool.tile([128, C], DT, tag="d_ss")
        nc.sync.dma_start(d_ss[:], ssel[:])
        sselb = pool.tile([NB, NCH * C], DT)
        nc.sync.dma_start(sselb[:], d_ss[:].rearrange("(k b) c -> b k c", k=NCH))
        sfin = pool.tile([NB, C], DT)
        nc.vector.tensor_reduce(sfin[:],
                                sselb[:].rearrange("b (k c) -> b c k", k=NCH),
                                AX.X, A.add)
        nc.vector.tensor_tensor(sfin[:], sfin[:], endr_t[0:NB, :], A.add)
        mxf = pool.tile([NB, 1], DT)
        nc.vector.tensor_reduce(mxf[:], sfin[:], AX.X, A.max)
        eqf = pool.tile([NB, C], DT)
        nc.vector.tensor_scalar(eqf[:], sfin[:], mxf[:], None, A.is_ge)
        vf = pool.tile([NB, C], DT)
        nc.vector.scalar_tensor_tensor(vf[:], eqf[:], float(ENC), io81_t[0:NB, 0:C],
                                       A.mult, A.subtract)
        lastenc = pool.tile([NB, 1], DT)
        nc.vector.tensor_reduce(lastenc[:], vf[:], AX.X, A.max)
        if DEBUG:
            nc.sync.dma_start(dbg["lastenc"], lastenc[:])

        ent128 = pool.tile([128, 1], DT)
        nc.vector.memset(ent128[:], 0.0)
        decf = pool.tile([128, CH], DT)
        entb = pool.tile([NB, 1], DT, tag="entb")
        nc.vector.tensor_copy(entb[:], lastenc[:])
        for k in range(NCH - 1, -1, -1):
            nc.sync.dma_start(ent128[32 * k:32 * (k + 1), :], entb[:])
            ohe = pool.tile([128, C], DT, tag="ohe")
            nc.vector.tensor_scalar(ohe[:], ioenc_t[:], ent128[:], None, A.is_equal)
            selw = pool.tile([128, CH * C], DT, tag="selw")
            nc.vector.tensor_tensor(
                selw[:].rearrange("p (i j) -> p i j", i=CH), tr_ij,
                ohe[:].unsqueeze(1).to_broadcast((128, CH, C)), A.mult)
            dsel = pool.tile([128, CH], DT, tag="dsel")
            nc.vector.tensor_reduce(dsel[:],
                                    selw[:].rearrange("p (i j) -> p i j", i=CH),
                                    AX.X, A.add)
            nc.vector.copy_predicated(decf[:], _bc(rowm_t[k][:], (128, CH)), dsel[:])
            nc.sync.dma_start(entb[:], dsel[32 * k:32 * (k + 1), 0:1])
        nc.vector.tensor_scalar(decf[:], decf[:], float(-ENC), -1.0, A.add, op1=A.mult)
        nc.vector.tensor_tensor(decf[:], decf[:], mAf_t[:], A.mult)
        if DEBUG:
            nc.sync.dma_start(dbg["decf"], decf[:])
        deci = pool.tile([128, CH], DI)
        nc.vector.tensor_copy(deci[:], decf[:])
        for k in range(NCH):
            nc.sync.dma_start(o_dec[:, CH * k:CH * (k + 1)],
                              deci[32 * k:32 * (k + 1), :])

        # ---------- P9: gold numerator + llh partial ----------
        junk = pool.tile([NB, C * L], DT, tag="junk")
        emp = pool.tile([NB, 1], DT)
        nc.vector.scalar_tensor_tensor(junk[:], em_gold[:], 1.0, w1_t[:],
                                       A.mult, A.mult, accum_out=emp[:])
        tw_ = pool.tile([NB, C2], DT, tag="tw_")
        nc.vector.tensor_tensor(tw_[:], counts_t[:], tflat_t[:], A.mult)
        tp = pool.tile([NB, 1], DT)
        nc.vector.tensor_reduce(tp[:], tw_[:], AX.X, A.add)
        sw_ = pool.tile([NB, C], DT, tag="sw_")
        nc.vector.tensor_tensor(sw_[:], ohst_t[:], startr_t[0:NB, :], A.mult)
        sp = pool.tile([NB, 1], DT)
        nc.vector.tensor_reduce(sp[:], sw_[:], AX.X, A.add)
        ew_ = pool.tile([NB, C], DT, tag="ew_")
        nc.vector.tensor_tensor(ew_[:], ohen_t[:], endr_t[0:NB, :], A.mult)
        ep = pool.tile([NB, 1], DT)
        nc.vector.tensor_reduce(ep[:], ew_[:], AX.X, A.add)
        num = pool.tile([NB, 1], DT)
        nc.vector.tensor_tensor(num[:], emp[:], tp[:], A.add)
        nc.vector.tensor_tensor(num[:], num[:], sp[:], A.add)
        nc.vector.tensor_tensor(num[:], num[:], ep[:], A.add)
        if DEBUG:
            nc.sync.dma_start(dbg["num"], num[:])
        diff = pool.tile([NB, 1], DT)
        nc.vector.tensor_tensor(diff[:], num[:], logZ[:], A.subtract)
        acc_ps = ppool2.tile([1, 1], DT, tag="accps")
        nc.tensor.matmul(acc_ps[:], diff[:], ones32_t[:], start=True, stop=True)
        acc = pool.tile([1, 1], DT)
        nc.scalar.copy(acc[:], acc_ps[:])
        nc.sync.dma_start(o_llh, acc[:])

    nc.compile()
    return nc


def host_prep(inputs):
    feats = np.asarray(inputs["feats"], f32)
    W = np.asarray(inputs["W_ff"], f32)
    b_ff = np.asarray(inputs["b_ff"], f32)
    start = np.asarray(inputs["start_transitions"], f32)
    end = np.asarray(inputs["end_transitions"], f32)
    T = np.asarray(inputs["transitions"], f32)
    tags = np.asarray(inputs["tags"])
    lengths = np.asarray(inputs["lengths"])

    expT = np.exp(T).astype(f32)
    rep = lambda a: np.ascontiguousarray(
        np.tile(np.asarray(a, f32).reshape(1, -1), (128, 1)))
    tt81 = rep(T.T.reshape(-1))
    ett81 = rep(expT.T.reshape(-1))
    iota81 = rep(np.tile(np.arange(C, dtype=f32), C))
    iotaenc = rep(ENC - np.arange(C, dtype=f32))
    expend = rep(np.exp(end))
    endr = rep(end)
    startr = rep(start)
    expstart = rep(np.exp(start))
    w_re = np.ascontiguousarray(
        W.reshape(KCH, 128, C).transpose(1, 0, 2).reshape(128, KCH * C)).astype(f32)

    shared = dict(
        w_re=w_re, bff=np.ascontiguousarray(b_ff.reshape(C, 1)).astype(f32),
        tt81=tt81, ett81=ett81, iota81=iota81, iotaenc=iotaenc,
        expend=expend, endr=endr, startr=startr, expstart=expstart,
        tflat=np.ascontiguousarray(np.tile(T.reshape(1, -1), (NB, 1))).astype(f32),
        ones32=np.ones((NB, 1), f32), onesN=np.ones((1, 512), f32),
    )

    in_maps = []
    bidx = np.arange(NB)
    for c in range(M):
        sl = slice(c * NB, (c + 1) * NB)
        fe = feats[sl]
        tg = tags[sl].astype(np.int64)
        ln = lengths[sl].astype(np.int64)
        # chunk (v,g): featsT[h, 512*(8v+g) + 128*b4 + t'] = fe[4g+b4, 128v+t', h]
        fw = fe.reshape(8, 4, NCH, CH, H).transpose(4, 2, 0, 1, 3).reshape(H, L * NB)
        featsT = np.ascontiguousarray(fw).astype(f32)

        m = (np.arange(L)[None, :] < ln[:, None])
        ohL = np.zeros((128, CH), f32)
        ohLany = np.zeros((128, 1), f32)
        mInv = np.zeros((128, CH), np.uint32)
        mAf = np.zeros((128, CH), f32)
        ch0m = np.zeros((128, 1), np.uint32)
        ch0m[0:32] = 1
        rowms = []
        for ch in range(NCH):
            t0 = ch * CH
            for b in range(NB):
                p = 32 * ch + b
                lm1 = int(ln[b]) - 1
                if t0 <= lm1 < t0 + CH:
                    ohL[p, lm1 - t0] = 1.0
                    ohLany[p, 0] = 1.0
                tv = np.arange(t0 + 1, t0 + CH + 1)
                mInv[p, :] = (tv >= ln[b]).astype(np.uint32)
                mAf[p, :] = m[b, t0:t0 + CH].astype(f32)
            rm = np.zeros((128, 1), np.uint32)
            rm[32 * ch:32 * (ch + 1)] = 1
            rowms.append(rm)
        w1 = np.zeros((NB, C, L), f32)
        for t in range(L):
            w1[bidx, tg[:, t], t] = m[:, t].astype(f32)
        counts = np.zeros((NB, C2), f32)
        for b in range(NB):
            for t in range(1, int(ln[b])):
                counts[b, tg[b, t - 1] * C + tg[b, t]] += 1
        ohst = np.zeros((NB, C), f32)
        ohst[bidx, tg[:, 0]] = 1
        ohen = np.zeros((NB, C), f32)
        ohen[bidx, tg[bidx, ln - 1]] = 1

        im = dict(shared)
        im.update(featsT=featsT, ohL=ohL, ohLany=ohLany, mInv=mInv, mAf=mAf,
                  ch0m=ch0m, w1=np.ascontiguousarray(w1.reshape(NB, C * L)),
                  counts=counts, ohst=ohst, ohen=ohen)
        for k in range(NCH):
            im[f"rowm{k}"] = rowms[k]
        in_maps.append(im)
    return in_maps


_prog_cache = {}


def get_program():
    if "nc" not in _prog_cache:
        _prog_cache["nc"] = build_program()
    return _prog_cache["nc"]


def _install_ntff_hook():
    """Provide antenv.axon_hooks via ctypes on images that lack it."""
    import types
    import ctypes
    import contextlib
    try:
        from antenv.axon_hooks import get_axon_ntff_profile_hook  # noqa: F401
        return
    except ImportError:
        pass
    try:
        lib = ctypes.CDLL("/opt/axon/libaxon_pjrt.so")
        if not hasattr(lib, "axon_start_nrt_profile"):
            return
    except OSError:
        return
    lib.axon_start_nrt_profile.argtypes = [ctypes.POINTER(ctypes.c_int64),
                                           ctypes.c_size_t]
    lib.axon_start_nrt_profile.restype = ctypes.c_int64
    lib.axon_stop_nrt_profile.argtypes = [ctypes.c_char_p]
    lib.axon_stop_nrt_profile.restype = ctypes.c_int64

    @contextlib.contextmanager
    def _hook(output_dir, device_ids):
        import jax
        jax.devices()
        if device_ids:
            ids = (ctypes.c_int64 * len(device_ids))(*device_ids)
            rc = lib.axon_start_nrt_profile(ids, len(device_ids))
        else:
            rc = lib.axon_start_nrt_profile(None, 0)
        if rc != 0:
            raise RuntimeError(f"axon_start_nrt_profile rc={rc}")
        try:
            yield
        finally:
            n = lib.axon_stop_nrt_profile(str(output_dir).encode())
            print(f"ntff profile: {n} file(s) -> {output_dir}")

    mod = types.ModuleType("antenv.axon_hooks")
    mod.get_axon_ntff_profile_hook = lambda: _hook
    mod.set_axon_ntff_profile_hook = lambda h: None
    sys.modules["antenv.axon_hooks"] = mod


def kernel(**inputs):
    nc = get_program()
    in_maps = host_prep(inputs)
    if TRACE:
        _install_ntff_hook()
    res = run_bass_kernel_spmd(nc, in_maps, list(range(M)), trace=TRACE)
    llh = np.sum([r["llh_part"][0, 0] for r in res.results], dtype=f32) / f32(B)
    decoded = np.concatenate([r["decoded"] for r in res.results], 0).astype(np.int32)
    kernel.last_results = res
    kernel.last_exec_time_ns = getattr(res, "exec_time_ns", None)
    return np.float32(llh), decoded
